# revision 31
# baseline (speedup 1.0000x reference)
"""BlockSparseThresLinear Trainium2 kernel.

Problem (hardcoded): x (128,1,4096) f16, weight (4096,11008) f16, bias (11008,) f16.
  BLOCK_M=16, BLOCK_K=64, THRES=0.8: per (16,64) block of x.reshape(128,4096),
  mask = mean(|block|, fp32) > 0.8; y = (x * mask_expanded) @ weight + bias.

Sharding: weight/bias column-sharded across 8 cores (1376 cols each); x
replicated; each core computes its output slice independently; host concats.

Per-core device pipeline (memory-bound: the 11.27MB W slice stream is the
roofline; cost-model total ~42us vs ~35us pure-DMA floor):
  - W streams on the sync/HWDGE queue only; x in 8 chunk tiles (first on
    the scalar/HWDGE queue, rest on gpsimd/SWDGE) so x never delays W.
  - per x chunk: DVE abs+sum over 64-wide blocks -> bsum [128,8] f32;
    PE matmul with block-diag GG^T (host input) sums each 16-row group
    (broadcast to all rows); DVE is_gt 819.2 -> maskrow {0,1} f16.
  - per K-chunk kc: DVE mul x*mask (step-0 broadcast AP), PE transpose
    (identity from host) -> PSUM f16, DVE copy -> xmT (deep pool so all
    transposes complete ahead of the W stream), 3 PE matmuls accumulate
    per-slice PSUM tiles [128,{512,512,352}] += xmT.T @ w_kc.
  - last two K-chunks stream slice-major so each output slice finishes
    (gemm -> DVE psum copy -> DMA out) while later slices still stream.
  - No ACT compute at all: keeps the scalar queue a pure DMA dispatcher
    (no LoadActFuncSet table load at the head).
"""

import numpy as np

M = 128
K = 4096
N_FULL = 11008
N_CORES = 8
NPC = N_FULL // N_CORES  # 1376
KC = K // 128  # 32 chunks
THRES_SUM = 819.2  # 0.8 * 1024 (exact in fp32: matches (sum/1024) > 0.8f)

_STATE = {}


def _build(bias_nonzero: bool, loop_reps: int = 1, variant: str = ""):
    from contextlib import ExitStack

    import concourse.bacc as bacc
    import concourse.bass as bass
    import concourse.mybir as mybir
    import concourse.tile as tile

    f16 = mybir.dt.float16
    f32 = mybir.dt.float32

    nc = bacc.Bacc(
        "TRN2",
        target_bir_lowering=False,
        debug=False,
        enable_asserts=False,
        num_devices=N_CORES,
    )

    if variant == "xstrided":
        x = nc.dram_tensor("x", [M, K], f16, kind="ExternalInput").ap()
    else:
        # chunk-major x layout (host repacks): each x chunk DMA reads a
        # contiguous 128KB region -- HW-measured ~2us faster than strided
        x = nc.dram_tensor("x", [K // 512, M, 512], f16, kind="ExternalInput").ap()
    w = nc.dram_tensor("w", [K, NPC], f16, kind="ExternalInput").ap()
    b = nc.dram_tensor("b", [1, NPC], f16, kind="ExternalInput").ap()
    gg = nc.dram_tensor("gg", [M, M], f32, kind="ExternalInput").ap()
    idin = nc.dram_tensor("idin", [128, 128], f16, kind="ExternalInput").ap()
    if variant == "tp":
        # three contiguous output tensors (host concatenates) -- avoids
        # strided DRAM writes on the critical tail
        youts = [
            nc.dram_tensor(f"y{i}", [M, wd], f16, kind="ExternalOutput").ap()
            for i, wd in enumerate((512, 512, NPC - 1024))
        ]
    else:
        y = nc.dram_tensor("y", [M, NPC], f16, kind="ExternalOutput").ap()

    # Output N split into PSUM-bank-sized slices (<=512 fp32 per bank).
    n_slices = [(0, 512), (512, 1024), (1024, NPC)]

    XCH = 8
    xw = K // XCH  # 512 cols = 8 blocks = 4 K-chunks per x chunk
    KC_G = KC // XCH

    with tile.TileContext(nc) as tc, ExitStack() as ctx:
        if loop_reps > 1:
            # benchmark-only: repeat the whole pipeline on-device so
            # differential wall timing can resolve the per-iteration time
            ctx.enter_context(tc.For_i(0, loop_reps, 1))
        singles = ctx.enter_context(tc.tile_pool(name="singles", bufs=1))
        wbufs = {"wb8": 8, "wb12": 12, "wb28": 28}.get(variant, 20)
        wpool = ctx.enter_context(tc.tile_pool(name="wpool", bufs=wbufs))
        xmpool = ctx.enter_context(tc.tile_pool(name="xmpool", bufs=8))
        xmtpool = ctx.enter_context(tc.tile_pool(name="xmtpool", bufs=KC))
        mrpool = ctx.enter_context(tc.tile_pool(name="mrpool", bufs=4))
        outpool = ctx.enter_context(tc.tile_pool(name="outpool", bufs=1))
        wlpool = ctx.enter_context(tc.tile_pool(name="wlpool", bufs=2))
        ps_t = ctx.enter_context(tc.tile_pool(name="ps_t", bufs=3, space="PSUM"))
        ps_y = ctx.enter_context(tc.tile_pool(name="ps_y", bufs=1, space="PSUM"))
        ps_m = ctx.enter_context(tc.tile_pool(name="ps_m", bufs=2, space="PSUM"))

        # Prologue DMAs split across independent dispatch resources: x0/gg/
        # ident on the scalar/HWDGE queue, bulk x chunks on gpsimd/SWDGE
        # (parallel dispatcher), so the sync/HWDGE queue carries nothing but
        # the W stream.
        xtiles = []
        for c in range(XCH):
            xsb = singles.tile([M, xw], f16, tag=f"xsb{c}")
            eng = nc.scalar if c == 0 else nc.gpsimd
            xin = x[:, c * xw : (c + 1) * xw] if variant == "xstrided" else x[c]
            eng.dma_start(out=xsb[:], in_=xin)
            xtiles.append(xsb)

        ggs = singles.tile([M, M], f32)
        nc.scalar.dma_start(out=ggs[:], in_=gg[:])
        ident = singles.tile([128, 128], f16)
        nc.scalar.dma_start(out=ident[:], in_=idin[:])

        if bias_nonzero:
            bias_b = singles.tile([M, NPC], f16)
            bcast = bass.AP(tensor=b.tensor, offset=b.offset, ap=[[0, M], b.ap[1]])
            nc.sync.dma_start(out=bias_b[:], in_=bcast)

        ypsums = {}
        for i, (lo, hi) in enumerate(n_slices):
            yps_tile = ps_y.tile([M, hi - lo], f32, tag=f"ypsum{i}")
            ypsums[lo] = yps_tile
        ysb = outpool.tile([M, NPC], f16)

        def emit_out_range(pk, a, bnd):
            # PSUM[pk] sub-range -> f16 SBUF (+bias) on DVE, then DMA out.
            # DVE-only keeps ACT a pure DMA-dispatch queue (no LoadActFuncSet
            # table load blocking the x0 dispatch).
            if bias_nonzero:
                nc.vector.tensor_tensor(
                    out=ysb[:, a:bnd],
                    in0=ypsums[pk][:, a - pk : bnd - pk],
                    in1=bias_b[:, a:bnd],
                    op=mybir.AluOpType.add,
                )
            else:
                nc.vector.tensor_copy(
                    out=ysb[:, a:bnd], in_=ypsums[pk][:, a - pk : bnd - pk]
                )
            # middle slice on the scalar queue so y dispatches overlap
            eng = nc.scalar if a == 512 else nc.sync
            if variant == "tp":
                eng.dma_start(out=youts[[0, 512, 1024].index(pk)][:, a - pk :], in_=ysb[:, a:bnd])
            else:
                eng.dma_start(out=y[:, a:bnd], in_=ysb[:, a:bnd])

        xmt_tail = {}
        for c in range(XCH):
            xsb = xtiles[c]
            nbl = xw // 64  # 16 blocks
            bsum = mrpool.tile([M, nbl], f32, tag="bsum")
            nc.vector.tensor_reduce(
                out=bsum[:],
                in_=xsb[:].rearrange("p (b q) -> p b q", q=64),
                axis=mybir.AxisListType.X,
                op=mybir.AluOpType.add,
                apply_absolute_value=True,
            )
            gsum = ps_m.tile([M, nbl], f32)
            nc.tensor.matmul(gsum[:], lhsT=ggs[:], rhs=bsum[:], start=True, stop=True)
            maskrow = mrpool.tile([M, nbl], f16, tag="maskrow")
            nc.vector.tensor_scalar(
                out=maskrow[:],
                in0=gsum[:],
                scalar1=float(THRES_SUM),
                scalar2=None,
                op0=mybir.AluOpType.is_gt,
            )

            wsb2 = None
            for j in range(KC_G):
                kc = c * KC_G + j
                tailk = kc >= KC - 2
                if not tailk and variant == "wpair":
                    # one DMA per K-chunk PAIR (704KB) halves W DMA count
                    if j % 2 == 0:
                        wsb2 = wpool.tile([128, 2, NPC], f16, tag="wsb2")
                        nc.sync.dma_start(
                            out=wsb2[:],
                            in_=w[kc * 128 : (kc + 2) * 128, :].rearrange(
                                "(a p) n -> p a n", p=128
                            ),
                        )
                    wsb = wsb2[:, j % 2, :]
                elif not tailk:
                    wsb_t = wpool.tile([128, NPC], f16, tag="wsb")
                    weng = nc.scalar if (variant == "w2q" and kc % 2) else nc.sync
                    weng.dma_start(
                        out=wsb_t[:], in_=w[kc * 128 : (kc + 1) * 128, :]
                    )
                    wsb = wsb_t[:]

                xm = xmpool.tile([128, 128], f16)
                mview = maskrow[:, 2 * j : 2 * j + 2].unsqueeze(2).broadcast_to(
                    [128, 2, 64]
                )
                nc.vector.tensor_tensor(
                    out=xm[:].rearrange("p (b q) -> p b q", q=64),
                    in0=xsb[:, j * 128 : (j + 1) * 128].rearrange(
                        "p (b q) -> p b q", q=64
                    ),
                    in1=mview,
                    op=mybir.AluOpType.mult,
                )

                pst = ps_t.tile([128, 128], f16)
                nc.tensor.transpose(pst[:], xm[:], ident[:])
                xmt = xmtpool.tile([128, 128], f16)
                nc.vector.tensor_copy(out=xmt[:], in_=pst[:])

                if not tailk:
                    for lo, hi in n_slices:
                        nc.tensor.matmul(
                            ypsums[lo][:],
                            lhsT=xmt[:],
                            rhs=wsb[:, lo:hi],
                            start=(kc == 0),
                            stop=False,
                        )
                else:
                    xmt_tail[kc] = xmt
                    if kc == KC - 1:
                        tail_pieces = [
                            (0, 0, 512),
                            (512, 512, 1024),
                            (1024, 1024, NPC),
                        ]
                        if variant == "tp":
                            # full contiguous tail-chunk DMAs (no strided
                            # piece reads); per-slice gemm/copy/out pipeline
                            wtl = {}
                            for kk in (KC - 2, KC - 1):
                                wt = wlpool.tile(
                                    [128, NPC], f16, tag=f"wt{kk % 2}"
                                )
                                nc.sync.dma_start(
                                    out=wt[:],
                                    in_=w[kk * 128 : (kk + 1) * 128, :],
                                )
                                wtl[kk] = wt
                            for pk, a, bnd in tail_pieces:
                                for kk in (KC - 2, KC - 1):
                                    nc.tensor.matmul(
                                        ypsums[pk][:, a - pk : bnd - pk],
                                        lhsT=xmt_tail[kk][:],
                                        rhs=wtl[kk][:, a:bnd],
                                        start=False,
                                        stop=(kk == KC - 1),
                                    )
                                emit_out_range(pk, a, bnd)
                        else:
                            # Final two K-chunks stream slice-major: each
                            # slice's last gemms -> psum copy -> output DMA
                            # pipeline while later slices still stream.
                            for pk, a, bnd in tail_pieces:
                                for kk in (KC - 2, KC - 1):
                                    wl = wlpool.tile(
                                        [128, bnd - a], f16, tag=f"wl{a}_{kk % 2}"
                                    )
                                    wleng = (
                                        nc.scalar
                                        if (variant == "w2q" and kk % 2)
                                        else nc.sync
                                    )
                                    wleng.dma_start(
                                        out=wl[:],
                                        in_=w[kk * 128 : (kk + 1) * 128, a:bnd],
                                    )
                                    nc.tensor.matmul(
                                        ypsums[pk][:, a - pk : bnd - pk],
                                        lhsT=xmt_tail[kk][:],
                                        rhs=wl[:],
                                        start=False,
                                        stop=(kk == KC - 1),
                                    )
                                emit_out_range(pk, a, bnd)

    nc.compile()
    return nc


def _build_v2(bias_nonzero: bool, loop_reps: int = 1, variant: str = "v2"):
    """Group-DMA pipeline: W host-pretiled so partition p holds row kc*128+p
    of every K-chunk; the stream is 7 DMAs of 4 K-chunks (desc 11008B, HW
    plateau ~34.8us for the 11.27MB) + a fine-grained tail (2+1+3-piece) so
    the last W bytes feed a short matmul->copy->y chain. x loads as ONE flat
    [128,4096] DMA ("v2": scalar queue; "v2f": fused into W group 0). One
    DVE reduce computes all 64 block sums; one PE matmul + is_gt gives the
    full [128,64] mask. y emits per-slice on the scalar queue ("...1": one
    [128,1376] DMA at the end instead).
    """
    from contextlib import ExitStack

    import concourse.bacc as bacc
    import concourse.bass as bass
    import concourse.mybir as mybir
    import concourse.tile as tile

    f16 = mybir.dt.float16
    f32 = mybir.dt.float32

    fused_x = variant.startswith("v2f")
    one_y = "1" in variant[2:]
    wbufs = 7 if "b7" in variant else 4

    nc = bacc.Bacc(
        "TRN2",
        target_bir_lowering=False,
        debug=False,
        enable_asserts=False,
        num_devices=N_CORES,
    )

    GS = 4  # K-chunks per W group DMA
    NG = KC // GS  # 8 groups; last group streams fine-grained
    WG = GS * NPC  # 5504 cols per full group
    wp_cols = KC * NPC + (K if fused_x else 0)
    wp = nc.dram_tensor("wp", [128, wp_cols], f16, kind="ExternalInput").ap()
    if not fused_x:
        xf = nc.dram_tensor("xf", [M, K], f16, kind="ExternalInput").ap()
    b = nc.dram_tensor("b", [1, NPC], f16, kind="ExternalInput").ap()
    gg = nc.dram_tensor("gg", [M, M], f32, kind="ExternalInput").ap()
    idin = nc.dram_tensor("idin", [128, 128], f16, kind="ExternalInput").ap()
    y = nc.dram_tensor("y", [M, NPC], f16, kind="ExternalOutput").ap()

    n_slices = [(0, 512), (512, 1024), (1024, NPC)]
    # offset of chunk kc's W columns inside the packed wp row
    xoff = K if fused_x else 0

    def wcol(kc, c0=0):
        return xoff + kc * NPC + c0

    with tile.TileContext(nc) as tc, ExitStack() as ctx:
        if loop_reps > 1:
            ctx.enter_context(tc.For_i(0, loop_reps, 1))
        singles = ctx.enter_context(tc.tile_pool(name="singles", bufs=1))
        xpool = ctx.enter_context(tc.tile_pool(name="xpool", bufs=2))
        wpool = ctx.enter_context(tc.tile_pool(name="wpool", bufs=wbufs))
        tlpool = ctx.enter_context(tc.tile_pool(name="tlpool", bufs=2))
        xmpool = ctx.enter_context(tc.tile_pool(name="xmpool", bufs=8))
        xmtpool = ctx.enter_context(tc.tile_pool(name="xmtpool", bufs=KC))
        mrpool = ctx.enter_context(tc.tile_pool(name="mrpool", bufs=2))
        outpool = ctx.enter_context(tc.tile_pool(name="outpool", bufs=2))
        ps_t = ctx.enter_context(tc.tile_pool(name="ps_t", bufs=3, space="PSUM"))
        ps_y = ctx.enter_context(tc.tile_pool(name="ps_y", bufs=1, space="PSUM"))
        ps_m = ctx.enter_context(tc.tile_pool(name="ps_m", bufs=2, space="PSUM"))

        # head loads on the scalar/ACT HWDGE queue; W owns the sync/SP queue
        ggs = singles.tile([M, M], f32)
        nc.scalar.dma_start(out=ggs[:], in_=gg[:])
        ident = singles.tile([128, 128], f16)
        nc.scalar.dma_start(out=ident[:], in_=idin[:])
        if bias_nonzero:
            bias_b = singles.tile([M, NPC], f16)
            bcast = bass.AP(tensor=b.tensor, offset=b.offset, ap=[[0, M], b.ap[1]])
            nc.scalar.dma_start(out=bias_b[:], in_=bcast)

        # W group DMAs: groups 0..6 coarse; group 7 = 2-chunk + 1-chunk +
        # three slice pieces of the final chunk (tail pipelining)
        wtiles = {}
        if fused_x:
            g0 = xpool.tile([128, K + WG], f16, tag="g0")
            nc.sync.dma_start(out=g0[:], in_=wp[:, : K + WG])
            xsb = g0[:, :K]
            wtiles[0] = g0
        else:
            xsb_t = xpool.tile([M, K], f16, tag="xsb")
            nc.scalar.dma_start(out=xsb_t[:], in_=xf[:])
            xsb = xsb_t[:]
            w0 = wpool.tile([128, WG], f16, tag="wg")
            nc.sync.dma_start(out=w0[:], in_=wp[:, xoff : xoff + WG])
            wtiles[0] = w0
        for g in range(1, NG - 1):
            wg = wpool.tile([128, WG], f16, tag="wg")
            nc.sync.dma_start(
                out=wg[:], in_=wp[:, wcol(g * GS) : wcol((g + 1) * GS)]
            )
            wtiles[g] = wg
        w2829 = tlpool.tile([128, 2 * NPC], f16, tag="w2829")
        nc.sync.dma_start(out=w2829[:], in_=wp[:, wcol(28) : wcol(30)])
        w30 = tlpool.tile([128, NPC], f16, tag="w30")
        nc.sync.dma_start(out=w30[:], in_=wp[:, wcol(30) : wcol(31)])
        wl31 = {}
        for lo, hi in n_slices:
            wl = tlpool.tile([128, hi - lo], f16, tag=f"wl31_{lo}")
            nc.sync.dma_start(out=wl[:], in_=wp[:, wcol(31, lo) : wcol(31, hi)])
            wl31[lo] = wl

        # mask: one reduce over all 64 blocks, one PE group-sum, one is_gt
        bsum = mrpool.tile([M, K // 64], f32, tag="bsum")
        nc.vector.tensor_reduce(
            out=bsum[:],
            in_=xsb.rearrange("p (b q) -> p b q", q=64),
            axis=mybir.AxisListType.X,
            op=mybir.AluOpType.add,
            apply_absolute_value=True,
        )
        gsum = ps_m.tile([M, K // 64], f32)
        nc.tensor.matmul(gsum[:], lhsT=ggs[:], rhs=bsum[:], start=True, stop=True)
        maskrow = mrpool.tile([M, K // 64], f16, tag="maskrow")
        nc.vector.tensor_scalar(
            out=maskrow[:],
            in0=gsum[:],
            scalar1=float(THRES_SUM),
            scalar2=None,
            op0=mybir.AluOpType.is_gt,
        )

        ypsums = {}
        for i, (lo, hi) in enumerate(n_slices):
            yps_tile = ps_y.tile([M, hi - lo], f32, tag=f"ypsum{i}")
            ypsums[lo] = yps_tile
        ysb = outpool.tile([M, NPC], f16)

        def make_xmt(kc):
            xm = xmpool.tile([128, 128], f16)
            mview = maskrow[:, 2 * kc : 2 * kc + 2].unsqueeze(2).broadcast_to(
                [128, 2, 64]
            )
            nc.vector.tensor_tensor(
                out=xm[:].rearrange("p (b q) -> p b q", q=64),
                in0=xsb[:, kc * 128 : (kc + 1) * 128].rearrange(
                    "p (b q) -> p b q", q=64
                ),
                in1=mview,
                op=mybir.AluOpType.mult,
            )
            pst = ps_t.tile([128, 128], f16)
            nc.tensor.transpose(pst[:], xm[:], ident[:])
            xmt = xmtpool.tile([128, 128], f16)
            nc.vector.tensor_copy(out=xmt[:], in_=pst[:])
            return xmt

        def emit_y(pk, a, bnd):
            if bias_nonzero:
                nc.vector.tensor_tensor(
                    out=ysb[:, a:bnd],
                    in0=ypsums[pk][:, a - pk : bnd - pk],
                    in1=bias_b[:, a:bnd],
                    op=mybir.AluOpType.add,
                )
            else:
                nc.vector.tensor_copy(
                    out=ysb[:, a:bnd], in_=ypsums[pk][:, a - pk : bnd - pk]
                )
            if not one_y:
                nc.scalar.dma_start(out=y[:, a:bnd], in_=ysb[:, a:bnd])

        xmts = {}
        for kc in range(KC):
            xmts[kc] = make_xmt(kc)
            if kc < 28:
                g, j = kc // GS, kc % GS
                wv = wtiles[g][:, (xoff if fused_x and g == 0 else 0) :]
                for lo, hi in n_slices:
                    nc.tensor.matmul(
                        ypsums[lo][:],
                        lhsT=xmts[kc][:],
                        rhs=wv[:, j * NPC + lo : j * NPC + hi],
                        start=(kc == 0),
                        stop=False,
                    )
            elif kc in (28, 29):
                for lo, hi in n_slices:
                    nc.tensor.matmul(
                        ypsums[lo][:],
                        lhsT=xmts[kc][:],
                        rhs=w2829[:, (kc - 28) * NPC + lo : (kc - 28) * NPC + hi],
                        start=False,
                        stop=False,
                    )
            elif kc == 30:
                for lo, hi in n_slices:
                    nc.tensor.matmul(
                        ypsums[lo][:],
                        lhsT=xmts[kc][:],
                        rhs=w30[:, lo:hi],
                        start=False,
                        stop=False,
                    )
            else:
                for lo, hi in n_slices:
                    nc.tensor.matmul(
                        ypsums[lo][:],
                        lhsT=xmts[kc][:],
                        rhs=wl31[lo][:],
                        start=False,
                        stop=True,
                    )
                    emit_y(lo, lo, hi)
        if one_y:
            nc.scalar.dma_start(out=y[:], in_=ysb[:])

    nc.compile()
    return nc


def _build_v3(bias_nonzero: bool, loop_reps: int = 1, variant: str = "v3"):
    """Baseline's fine-grained per-chunk compute pipeline (x in 8 chunk DMAs,
    per-chunk mask chain, deep xmt pool) with the W stream restructured into
    pretiled 4-K-chunk group DMAs (desc 11008B — HW plateau ~34.8us vs 36.2us
    for 32 single-chunk DMAs). Tail: chunks 28-29 single-chunk DMAs, chunks
    30-31 slice-major pieces with per-slice y emission.

    variant flags after "v3": 'y' = all y DMAs on scalar queue (default
    baseline mix: s1 scalar, s0/s2 sync).
    """
    from contextlib import ExitStack

    import concourse.bacc as bacc
    import concourse.bass as bass
    import concourse.mybir as mybir
    import concourse.tile as tile

    f16 = mybir.dt.float16
    f32 = mybir.dt.float32

    y_scalar = "y" in variant[2:]

    nc = bacc.Bacc(
        "TRN2",
        target_bir_lowering=False,
        debug=False,
        enable_asserts=False,
        num_devices=N_CORES,
    )

    GS = 4
    WG = GS * NPC
    x = nc.dram_tensor("x", [K // 512, M, 512], f16, kind="ExternalInput").ap()
    wp = nc.dram_tensor("wp", [128, KC * NPC], f16, kind="ExternalInput").ap()
    b = nc.dram_tensor("b", [1, NPC], f16, kind="ExternalInput").ap()
    gg = nc.dram_tensor("gg", [M, M], f32, kind="ExternalInput").ap()
    idin = nc.dram_tensor("idin", [128, 128], f16, kind="ExternalInput").ap()
    y = nc.dram_tensor("y", [M, NPC], f16, kind="ExternalOutput").ap()

    n_slices = [(0, 512), (512, 1024), (1024, NPC)]

    def wcol(kc, c0=0):
        return kc * NPC + c0

    XCH = 8
    xw = K // XCH

    with tile.TileContext(nc) as tc, ExitStack() as ctx:
        if loop_reps > 1:
            ctx.enter_context(tc.For_i(0, loop_reps, 1))
        singles = ctx.enter_context(tc.tile_pool(name="singles", bufs=1))
        wpool = ctx.enter_context(tc.tile_pool(name="wpool", bufs=4))
        tlpool = ctx.enter_context(tc.tile_pool(name="tlpool", bufs=2))
        xmpool = ctx.enter_context(tc.tile_pool(name="xmpool", bufs=8))
        xmtpool = ctx.enter_context(tc.tile_pool(name="xmtpool", bufs=KC))
        mrpool = ctx.enter_context(tc.tile_pool(name="mrpool", bufs=4))
        outpool = ctx.enter_context(tc.tile_pool(name="outpool", bufs=1))
        ps_t = ctx.enter_context(tc.tile_pool(name="ps_t", bufs=3, space="PSUM"))
        ps_y = ctx.enter_context(tc.tile_pool(name="ps_y", bufs=1, space="PSUM"))
        ps_m = ctx.enter_context(tc.tile_pool(name="ps_m", bufs=2, space="PSUM"))

        # x chunks: first on scalar/HWDGE, rest on gpsimd/SWDGE (baseline)
        xtiles = []
        for c in range(XCH):
            xsb = singles.tile([M, xw], f16, tag=f"xsb{c}")
            eng = nc.scalar if c == 0 else nc.gpsimd
            eng.dma_start(out=xsb[:], in_=x[c])
            xtiles.append(xsb)

        ggs = singles.tile([M, M], f32)
        nc.scalar.dma_start(out=ggs[:], in_=gg[:])
        ident = singles.tile([128, 128], f16)
        nc.scalar.dma_start(out=ident[:], in_=idin[:])

        if bias_nonzero:
            bias_b = singles.tile([M, NPC], f16)
            bcast = bass.AP(tensor=b.tensor, offset=b.offset, ap=[[0, M], b.ap[1]])
            nc.scalar.dma_start(out=bias_b[:], in_=bcast)

        # W group DMAs for chunks 0..27 (7 groups of 4)
        wgroups = {}
        for g in range(7):
            wg_t = wpool.tile([128, WG], f16, tag="wg")
            nc.sync.dma_start(
                out=wg_t[:], in_=wp[:, wcol(g * GS) : wcol((g + 1) * GS)]
            )
            wgroups[g] = wg_t
        # tail: 28, 29 single chunks; 30-31 slice-major pieces
        wtail = {}
        for kk in (28, 29):
            wt = tlpool.tile([128, NPC], f16, tag=f"wt{kk}")
            nc.sync.dma_start(out=wt[:], in_=wp[:, wcol(kk) : wcol(kk + 1)])
            wtail[kk] = wt
        wl = {}
        for lo, hi in n_slices:
            for kk in (30, 31):
                wl_t = tlpool.tile([128, hi - lo], f16, tag=f"wl{lo}_{kk}")
                nc.sync.dma_start(
                    out=wl_t[:], in_=wp[:, wcol(kk, lo) : wcol(kk, hi)]
                )
                wl[(kk, lo)] = wl_t

        ypsums = {}
        for i, (lo, hi) in enumerate(n_slices):
            yps_tile = ps_y.tile([M, hi - lo], f32, tag=f"ypsum{i}")
            ypsums[lo] = yps_tile
        ysb = outpool.tile([M, NPC], f16)

        def emit_out_range(pk, a, bnd):
            if bias_nonzero:
                nc.vector.tensor_tensor(
                    out=ysb[:, a:bnd],
                    in0=ypsums[pk][:, a - pk : bnd - pk],
                    in1=bias_b[:, a:bnd],
                    op=mybir.AluOpType.add,
                )
            else:
                nc.vector.tensor_copy(
                    out=ysb[:, a:bnd], in_=ypsums[pk][:, a - pk : bnd - pk]
                )
            eng = nc.scalar if (y_scalar or a == 512) else nc.sync
            eng.dma_start(out=y[:, a:bnd], in_=ysb[:, a:bnd])

        xmt_all = {}
        for c in range(XCH):
            xsb = xtiles[c]
            nbl = xw // 64
            bsum = mrpool.tile([M, nbl], f32, tag="bsum")
            nc.vector.tensor_reduce(
                out=bsum[:],
                in_=xsb[:].rearrange("p (b q) -> p b q", q=64),
                axis=mybir.AxisListType.X,
                op=mybir.AluOpType.add,
                apply_absolute_value=True,
            )
            gsum = ps_m.tile([M, nbl], f32)
            nc.tensor.matmul(
                gsum[:], lhsT=ggs[:], rhs=bsum[:], start=True, stop=True
            )
            maskrow = mrpool.tile([M, nbl], f16, tag="maskrow")
            nc.vector.tensor_scalar(
                out=maskrow[:],
                in0=gsum[:],
                scalar1=float(THRES_SUM),
                scalar2=None,
                op0=mybir.AluOpType.is_gt,
            )

            for j in range(4):
                kc = c * 4 + j
                xm = xmpool.tile([128, 128], f16)
                mview = maskrow[:, 2 * j : 2 * j + 2].unsqueeze(2).broadcast_to(
                    [128, 2, 64]
                )
                nc.vector.tensor_tensor(
                    out=xm[:].rearrange("p (b q) -> p b q", q=64),
                    in0=xsb[:, j * 128 : (j + 1) * 128].rearrange(
                        "p (b q) -> p b q", q=64
                    ),
                    in1=mview,
                    op=mybir.AluOpType.mult,
                )
                pst = ps_t.tile([128, 128], f16)
                nc.tensor.transpose(pst[:], xm[:], ident[:])
                xmt = xmtpool.tile([128, 128], f16)
                nc.vector.tensor_copy(out=xmt[:], in_=pst[:])
                xmt_all[kc] = xmt

                if kc < 28:
                    wv = wgroups[kc // GS]
                    for lo, hi in n_slices:
                        nc.tensor.matmul(
                            ypsums[lo][:],
                            lhsT=xmt[:],
                            rhs=wv[:, (kc % GS) * NPC + lo : (kc % GS) * NPC + hi],
                            start=(kc == 0),
                            stop=False,
                        )
                elif kc in (28, 29):
                    for lo, hi in n_slices:
                        nc.tensor.matmul(
                            ypsums[lo][:],
                            lhsT=xmt[:],
                            rhs=wtail[kc][:, lo:hi],
                            start=False,
                            stop=False,
                        )
                elif kc == 31:
                    # slice-major: finish each slice then emit while later
                    # slices still stream
                    for lo, hi in n_slices:
                        for kk in (30, 31):
                            nc.tensor.matmul(
                                ypsums[lo][:],
                                lhsT=xmt_all[kk][:],
                                rhs=wl[(kk, lo)][:],
                                start=False,
                                stop=(kk == 31),
                            )
                        emit_out_range(lo, lo, hi)

    nc.compile()
    return nc


def _build_v4(bias_nonzero: bool, loop_reps: int = 1, variant: str = "v4"):
    """v3's fine-grained compute pipeline with x FUSED into the W stream:
    one packed DRAM tensor wx = [x0 | (x1|Wg0) | (x2|Wg1) | ... | (x7|Wg6) |
    W28..31]. Group g's single sync DMA (desc 12032B) delivers x chunk g+1
    one group ahead of its consumers; x0/gg/ident load tiny on scalar at the
    head. No separate x DMAs to lose DMA-pool arbitration to the W groups.

    flags after "v4": 'y' = all y DMAs on scalar (default: s1 scalar,
    s0/s2 sync). Variant "v5*": software-pipeline matmuls one group behind
    the mask/transpose chain so PE's in-order queue never stalls a
    transpose behind matmuls (breaks the per-chunk T->copy->mm latency
    round trip that capped the un-pipelined order at ~1.6us/chunk).
    """
    from contextlib import ExitStack

    import concourse.bacc as bacc
    import concourse.bass as bass
    import concourse.mybir as mybir
    import concourse.tile as tile

    f16 = mybir.dt.float16
    f32 = mybir.dt.float32

    y_scalar = "y" in variant[2:]
    pipelined = variant.startswith("v5")
    head_sync = "h" in variant[2:]

    nc = bacc.Bacc(
        "TRN2",
        target_bir_lowering=False,
        debug=False,
        enable_asserts=False,
        num_devices=N_CORES,
    )

    GS = 4
    WG = GS * NPC  # 5504
    GW = 512 + WG  # 6016 cols per fused group
    wx = nc.dram_tensor(
        "wx", [128, K + KC * NPC], f16, kind="ExternalInput"
    ).ap()
    b = nc.dram_tensor("b", [1, NPC], f16, kind="ExternalInput").ap()
    gg = nc.dram_tensor("gg", [M, M], f32, kind="ExternalInput").ap()
    idin = nc.dram_tensor("idin", [128, 128], f16, kind="ExternalInput").ap()
    y = nc.dram_tensor("y", [M, NPC], f16, kind="ExternalOutput").ap()

    n_slices = [(0, 512), (512, 1024), (1024, NPC)]
    TAIL0 = 512 + 7 * GW  # col offset of chunk 28

    def tailcol(kk, c0=0):
        return TAIL0 + (kk - 28) * NPC + c0

    with tile.TileContext(nc) as tc, ExitStack() as ctx:
        if loop_reps > 1:
            ctx.enter_context(tc.For_i(0, loop_reps, 1))
        singles = ctx.enter_context(tc.tile_pool(name="singles", bufs=1))
        x0pool = ctx.enter_context(tc.tile_pool(name="x0pool", bufs=2))
        wpool = ctx.enter_context(tc.tile_pool(name="wpool", bufs=8))
        tlpool = ctx.enter_context(tc.tile_pool(name="tlpool", bufs=2))
        xmpool = ctx.enter_context(tc.tile_pool(name="xmpool", bufs=8))
        xmtpool = ctx.enter_context(tc.tile_pool(name="xmtpool", bufs=KC))
        mrpool = ctx.enter_context(tc.tile_pool(name="mrpool", bufs=4))
        outpool = ctx.enter_context(tc.tile_pool(name="outpool", bufs=2))
        ps_t = ctx.enter_context(tc.tile_pool(name="ps_t", bufs=3, space="PSUM"))
        ps_y = ctx.enter_context(tc.tile_pool(name="ps_y", bufs=1, space="PSUM"))
        ps_m = ctx.enter_context(tc.tile_pool(name="ps_m", bufs=2, space="PSUM"))

        head_eng = nc.sync if head_sync else nc.scalar
        ggs = singles.tile([M, M], f32)
        head_eng.dma_start(out=ggs[:], in_=gg[:])
        ident = singles.tile([128, 128], f16)
        head_eng.dma_start(out=ident[:], in_=idin[:])
        x0 = x0pool.tile([M, 512], f16, tag="x0")
        head_eng.dma_start(out=x0[:], in_=wx[:, :512])

        if bias_nonzero:
            bias_b = singles.tile([M, NPC], f16)
            bcast = bass.AP(tensor=b.tensor, offset=b.offset, ap=[[0, M], b.ap[1]])
            nc.scalar.dma_start(out=bias_b[:], in_=bcast)

        # fused group DMAs: [x chunk g+1 | W chunks 4g..4g+3]
        wgroups = {}
        for g in range(7):
            wg_t = wpool.tile([128, GW], f16, tag="wg")
            nc.sync.dma_start(
                out=wg_t[:], in_=wx[:, 512 + g * GW : 512 + (g + 1) * GW]
            )
            wgroups[g] = wg_t
        wtail = {}
        for kk in (28, 29):
            wt = tlpool.tile([128, NPC], f16, tag=f"wt{kk}")
            nc.sync.dma_start(out=wt[:], in_=wx[:, tailcol(kk) : tailcol(kk + 1)])
            wtail[kk] = wt
        wl = {}
        for lo, hi in n_slices:
            for kk in (30, 31):
                wl_t = tlpool.tile([128, hi - lo], f16, tag=f"wl{lo}_{kk}")
                nc.sync.dma_start(
                    out=wl_t[:], in_=wx[:, tailcol(kk, lo) : tailcol(kk, hi)]
                )
                wl[(kk, lo)] = wl_t

        xtiles = [x0[:]] + [wgroups[g][:, :512] for g in range(7)]

        ypsums = {}
        for i, (lo, hi) in enumerate(n_slices):
            yps_tile = ps_y.tile([M, hi - lo], f32, tag=f"ypsum{i}")
            ypsums[lo] = yps_tile
        ysb = outpool.tile([M, NPC], f16)

        def emit_out_range(pk, a, bnd):
            if bias_nonzero:
                nc.vector.tensor_tensor(
                    out=ysb[:, a:bnd],
                    in0=ypsums[pk][:, a - pk : bnd - pk],
                    in1=bias_b[:, a:bnd],
                    op=mybir.AluOpType.add,
                )
            else:
                nc.vector.tensor_copy(
                    out=ysb[:, a:bnd], in_=ypsums[pk][:, a - pk : bnd - pk]
                )
            eng = nc.scalar if (y_scalar or a == 512) else nc.sync
            eng.dma_start(out=y[:, a:bnd], in_=ysb[:, a:bnd])

        xmt_all = {}

        def mask_and_transpose(c):
            xsb = xtiles[c]
            nbl = 8
            bsum = mrpool.tile([M, nbl], f32, tag="bsum")
            nc.vector.tensor_reduce(
                out=bsum[:],
                in_=xsb.rearrange("p (b q) -> p b q", q=64),
                axis=mybir.AxisListType.X,
                op=mybir.AluOpType.add,
                apply_absolute_value=True,
            )
            gsum = ps_m.tile([M, nbl], f32)
            nc.tensor.matmul(
                gsum[:], lhsT=ggs[:], rhs=bsum[:], start=True, stop=True
            )
            maskrow = mrpool.tile([M, nbl], f16, tag="maskrow")
            nc.vector.tensor_scalar(
                out=maskrow[:],
                in0=gsum[:],
                scalar1=float(THRES_SUM),
                scalar2=None,
                op0=mybir.AluOpType.is_gt,
            )
            for j in range(4):
                kc = c * 4 + j
                xm = xmpool.tile([128, 128], f16)
                mview = maskrow[:, 2 * j : 2 * j + 2].unsqueeze(2).broadcast_to(
                    [128, 2, 64]
                )
                nc.vector.tensor_tensor(
                    out=xm[:].rearrange("p (b q) -> p b q", q=64),
                    in0=xsb[:, j * 128 : (j + 1) * 128].rearrange(
                        "p (b q) -> p b q", q=64
                    ),
                    in1=mview,
                    op=mybir.AluOpType.mult,
                )
                pst = ps_t.tile([128, 128], f16)
                nc.tensor.transpose(pst[:], xm[:], ident[:])
                xmt = xmtpool.tile([128, 128], f16)
                nc.vector.tensor_copy(out=xmt[:], in_=pst[:])
                xmt_all[kc] = xmt

        def matmuls_for(kc):
            if kc < 28:
                wv = wgroups[kc // GS]
                for lo, hi in n_slices:
                    nc.tensor.matmul(
                        ypsums[lo][:],
                        lhsT=xmt_all[kc][:],
                        rhs=wv[
                            :,
                            512 + (kc % GS) * NPC + lo : 512
                            + (kc % GS) * NPC
                            + hi,
                        ],
                        start=(kc == 0),
                        stop=False,
                    )
            elif kc in (28, 29):
                for lo, hi in n_slices:
                    nc.tensor.matmul(
                        ypsums[lo][:],
                        lhsT=xmt_all[kc][:],
                        rhs=wtail[kc][:, lo:hi],
                        start=False,
                        stop=False,
                    )
            elif kc == 31:
                for lo, hi in n_slices:
                    for kk in (30, 31):
                        nc.tensor.matmul(
                            ypsums[lo][:],
                            lhsT=xmt_all[kk][:],
                            rhs=wl[(kk, lo)][:],
                            start=False,
                            stop=(kk == 31),
                        )
                    emit_out_range(lo, lo, hi)

        if pipelined:
            # group c's mask/transposes precede group c-1's matmuls in PE
            # program order, so a transpose never queues behind matmuls
            # whose W group hasn't streamed in yet
            for c in range(8):
                mask_and_transpose(c)
                if c >= 1:
                    for kc in range(4 * (c - 1), 4 * c):
                        matmuls_for(kc)
            for kc in range(28, 32):
                matmuls_for(kc)
        else:
            for c in range(8):
                mask_and_transpose(c)
                for kc in range(4 * c, 4 * c + 4):
                    matmuls_for(kc)

    nc.compile()
    return nc


def _build_v6(bias_nonzero: bool, loop_reps: int = 1, variant: str = "v6"):
    """Transpose-free pipeline: host supplies x PRE-TRANSPOSED (xT chunks of
    [128 k-part, 128 m]) packed into the fused W stream, so matmul lhsT comes
    straight from a DVE mask-multiply — no PE transposes, no PSUM round trip,
    no per-chunk copies. Masks are computed in transposed space per group:
    DVE reduce |xT| over 16-wide m-groups -> pbs [128, 4*8]; one PE matmul
    with BB = kron(eye(2), ones(64)) sums each k-block's 64 partitions ->
    mask_pre; is_gt -> maskT; DVE mult applies it. Stream layout per
    partition: [BBrow? no - BB separate f32 | xT0(chunks 0-3) | (xT(4g+4..7)
    | Wg) x7 | Wtail], head (BB, xT0) at the front of the sync queue.

    flags after "v6": 'y' = all y on scalar.
    """
    from contextlib import ExitStack

    import concourse.bacc as bacc
    import concourse.bass as bass
    import concourse.mybir as mybir
    import concourse.tile as tile

    f16 = mybir.dt.float16
    f32 = mybir.dt.float32

    y_scalar = "y" in variant[2:]
    piece_emit = "t" in variant[2:]  # piecewise copy+y after slice-wide stop
    fine_tail = "f" in variant[2:]  # chunks 24-29 as single-chunk DMAs
    act_copy = "a" in variant[2:]  # middle slice psum->sbuf copy on ACT

    nc = bacc.Bacc(
        "TRN2",
        target_bir_lowering=False,
        debug=False,
        enable_asserts=False,
        num_devices=N_CORES,
    )

    GS = 4
    WG = GS * NPC  # 5504
    GW = 512 + WG  # 6016: 4 xT chunks (4*128) + 4 W chunks
    HD = 768  # head: 256 cols of BB-as-f16-bytes + 512 cols xT chunks 0-3
    wx = nc.dram_tensor(
        "wx", [128, HD + K - 512 + KC * NPC], f16, kind="ExternalInput"
    ).ap()
    b = nc.dram_tensor("b", [1, NPC], f16, kind="ExternalInput").ap()
    y = nc.dram_tensor("y", [M, NPC], f16, kind="ExternalOutput").ap()

    n_slices = [(0, 512), (512, 1024), (1024, NPC)]
    TAIL0 = HD + 7 * GW  # col offset of W chunk 28

    def tailcol(kk, c0=0):
        return TAIL0 + (kk - 28) * NPC + c0

    with tile.TileContext(nc) as tc, ExitStack() as ctx:
        if loop_reps > 1:
            ctx.enter_context(tc.For_i(0, loop_reps, 1))
        singles = ctx.enter_context(tc.tile_pool(name="singles", bufs=1))
        x0pool = ctx.enter_context(tc.tile_pool(name="x0pool", bufs=2))
        wpool = ctx.enter_context(tc.tile_pool(name="wpool", bufs=8))
        tlpool = ctx.enter_context(tc.tile_pool(name="tlpool", bufs=2))
        xmtpool = ctx.enter_context(tc.tile_pool(name="xmtpool", bufs=KC))
        mrpool = ctx.enter_context(tc.tile_pool(name="mrpool", bufs=4))
        outpool = ctx.enter_context(tc.tile_pool(name="outpool", bufs=2))
        ps_y = ctx.enter_context(tc.tile_pool(name="ps_y", bufs=1, space="PSUM"))
        ps_m = ctx.enter_context(tc.tile_pool(name="ps_m", bufs=2, space="PSUM"))

        # head on sync so nothing loses DMA arbitration to the W groups;
        # BB rides as raw bytes in the f16 tile, bitcast back to f32
        head = x0pool.tile([128, HD], f16, tag="head")
        nc.sync.dma_start(out=head[:], in_=wx[:, :HD])
        bbs = head[:, :256].bitcast(f32)
        xt0 = head[:, 256:HD]
        if act_copy:
            # warmup so any ACT table load lands at the head, not the tail
            warm = singles.tile([128, 1], f16)
            nc.scalar.activation(
                out=warm[:], in_=head[:, :1],
                func=mybir.ActivationFunctionType.Copy,
            )

        if bias_nonzero:
            bias_b = singles.tile([M, NPC], f16)
            bcast = bass.AP(tensor=b.tensor, offset=b.offset, ap=[[0, M], b.ap[1]])
            nc.scalar.dma_start(out=bias_b[:], in_=bcast)

        ngroups = 6 if fine_tail else 7
        wgroups = {}
        for g in range(ngroups):
            wg_t = wpool.tile([128, GW], f16, tag="wg")
            nc.sync.dma_start(
                out=wg_t[:], in_=wx[:, HD + g * GW : HD + (g + 1) * GW]
            )
            wgroups[g] = wg_t
        wtail = {}
        if fine_tail:
            # group 6's region re-sliced: [xT(28-31) | W24] one DMA, then
            # W25..29 as singles (the host layout is unchanged)
            GRP6 = HD + 6 * GW
            t24 = tlpool.tile([128, 512 + NPC], f16, tag="t24")
            nc.sync.dma_start(out=t24[:], in_=wx[:, GRP6 : GRP6 + 512 + NPC])
            wtail[24] = t24[:, 512:]
            for kk in range(25, 30):
                wt = tlpool.tile([128, NPC], f16, tag=f"wt{kk}")
                nc.sync.dma_start(
                    out=wt[:],
                    in_=wx[
                        :, GRP6 + 512 + (kk - 24) * NPC : GRP6
                        + 512
                        + (kk - 23) * NPC
                    ],
                )
                wtail[kk] = wt[:]
        else:
            for kk in (28, 29):
                wt = tlpool.tile([128, NPC], f16, tag=f"wt{kk}")
                nc.sync.dma_start(
                    out=wt[:], in_=wx[:, tailcol(kk) : tailcol(kk + 1)]
                )
                wtail[kk] = wt[:]
        wl = {}
        for lo, hi in n_slices:
            for kk in (30, 31):
                wl_t = tlpool.tile([128, hi - lo], f16, tag=f"wl{lo}_{kk}")
                nc.sync.dma_start(
                    out=wl_t[:], in_=wx[:, tailcol(kk, lo) : tailcol(kk, hi)]
                )
                wl[(kk, lo)] = wl_t

        # xT source view for chunk group c (chunks 4c..4c+3)
        xtsrc = [xt0] + [wgroups[g][:, :512] for g in range(ngroups)]
        if fine_tail:
            xtsrc.append(t24[:, :512])

        ypsums = {}
        for i, (lo, hi) in enumerate(n_slices):
            yps_tile = ps_y.tile([M, hi - lo], f32, tag=f"ypsum{i}")
            ypsums[lo] = yps_tile
        ysb = outpool.tile([M, NPC], f16)

        def emit_out_range(pk, a, bnd):
            if bias_nonzero:
                nc.vector.tensor_tensor(
                    out=ysb[:, a:bnd],
                    in0=ypsums[pk][:, a - pk : bnd - pk],
                    in1=bias_b[:, a:bnd],
                    op=mybir.AluOpType.add,
                )
            elif act_copy and pk == 512:
                nc.scalar.activation(
                    out=ysb[:, a:bnd],
                    in_=ypsums[pk][:, a - pk : bnd - pk],
                    func=mybir.ActivationFunctionType.Copy,
                )
            else:
                nc.vector.tensor_copy(
                    out=ysb[:, a:bnd], in_=ypsums[pk][:, a - pk : bnd - pk]
                )
            eng = nc.scalar if (y_scalar or a == 512) else nc.sync
            eng.dma_start(out=y[:, a:bnd], in_=ysb[:, a:bnd])

        xmt_all = {}

        def masks_for_group(c):
            # chunks 4c..4c+3; xT in xtsrc[c]: [128, 4*128]
            xv = xtsrc[c]
            pbs = mrpool.tile([128, 32], f32, tag="pbs")
            nc.vector.tensor_reduce(
                out=pbs[:],
                in_=xv.rearrange("p (cg q) -> p cg q", q=16),
                axis=mybir.AxisListType.X,
                op=mybir.AluOpType.add,
                apply_absolute_value=True,
            )
            mask_pre = ps_m.tile([128, 32], f32)
            nc.tensor.matmul(
                mask_pre[:], lhsT=bbs, rhs=pbs[:], start=True, stop=True
            )
            maskt = mrpool.tile([128, 32], f16, tag="maskt")
            nc.vector.tensor_scalar(
                out=maskt[:],
                in0=mask_pre[:],
                scalar1=float(THRES_SUM),
                scalar2=None,
                op0=mybir.AluOpType.is_gt,
            )
            for j in range(4):
                kc = 4 * c + j
                xmt = xmtpool.tile([128, 128], f16)
                mview = maskt[:, 8 * j : 8 * j + 8].unsqueeze(2).broadcast_to(
                    [128, 8, 16]
                )
                nc.vector.tensor_tensor(
                    out=xmt[:].rearrange("p (g q) -> p g q", q=16),
                    in0=xv[:, j * 128 : (j + 1) * 128].rearrange(
                        "p (g q) -> p g q", q=16
                    ),
                    in1=mview,
                    op=mybir.AluOpType.mult,
                )
                xmt_all[kc] = xmt

        def matmuls_for(kc):
            first_single = 24 if fine_tail else 28
            if kc < first_single:
                wv = wgroups[kc // GS]
                for lo, hi in n_slices:
                    nc.tensor.matmul(
                        ypsums[lo][:],
                        lhsT=xmt_all[kc][:],
                        rhs=wv[
                            :,
                            512 + (kc % GS) * NPC + lo : 512
                            + (kc % GS) * NPC
                            + hi,
                        ],
                        start=(kc == 0),
                        stop=False,
                    )
            elif kc < 30:
                for lo, hi in n_slices:
                    nc.tensor.matmul(
                        ypsums[lo][:],
                        lhsT=xmt_all[kc][:],
                        rhs=wtail[kc][:, lo:hi],
                        start=False,
                        stop=False,
                    )
            elif kc == 31:
                for lo, hi in n_slices:
                    for kk in (30, 31):
                        nc.tensor.matmul(
                            ypsums[lo][:],
                            lhsT=xmt_all[kk][:],
                            rhs=wl[(kk, lo)][:],
                            start=False,
                            stop=(kk == 31),
                        )
                    if piece_emit and hi - lo > 256:
                        mid = lo + (hi - lo) // 2
                        emit_out_range(lo, lo, mid)
                        emit_out_range(lo, mid, hi)
                    else:
                        emit_out_range(lo, lo, hi)

        masks_for_group(0)
        for g in range(ngroups):
            masks_for_group(g + 1)
            for kc in range(4 * g, 4 * g + 4):
                matmuls_for(kc)
        if fine_tail:
            masks_for_group(7)
            for kc in range(24, 30):
                matmuls_for(kc)
        else:
            for kc in (28, 29):
                matmuls_for(kc)
        matmuls_for(31)

    nc.compile()
    return nc


def _make_in_maps_v6(x, weight, bias):
    x2d = np.asarray(x, dtype=np.float16).reshape(M, K)
    xt = np.ascontiguousarray(x2d.T)  # [K, M]; chunk kc = rows kc*128..
    wf = np.asarray(weight, dtype=np.float16)
    bf = np.asarray(bias, dtype=np.float16)
    bb = np.kron(np.eye(2, dtype=np.float32), np.ones((64, 64), np.float32))
    bb16 = np.ascontiguousarray(bb).view(np.float16)  # [128, 256] raw bytes
    xtc = xt.reshape(KC, 128, 128)  # [kc, k-part, m]
    in_maps = []
    for c in range(N_CORES):
        ws = wf[:, c * NPC : (c + 1) * NPC]
        wtiled = ws.reshape(KC, 128, NPC).transpose(1, 0, 2)  # [128, KC, NPC]
        parts = [bb16, xtc[0:4].transpose(1, 0, 2).reshape(128, 512)]
        for g in range(7):
            parts.append(
                xtc[4 * g + 4 : 4 * g + 8].transpose(1, 0, 2).reshape(128, 512)
            )
            parts.append(wtiled[:, 4 * g : 4 * g + 4, :].reshape(128, 4 * NPC))
        parts.append(wtiled[:, 28:32, :].reshape(128, 4 * NPC))
        wxm = np.ascontiguousarray(np.concatenate(parts, axis=1))
        in_maps.append(
            {
                "wx": wxm,
                "b": np.ascontiguousarray(bf[c * NPC : (c + 1) * NPC]).reshape(
                    1, NPC
                ),
            }
        )
    return in_maps


def _make_in_maps_v4(x, weight, bias):
    x2 = np.asarray(x, dtype=np.float16).reshape(M, K // 512, 512)
    wf = np.asarray(weight, dtype=np.float16)
    bf = np.asarray(bias, dtype=np.float16)
    gg = np.kron(np.eye(8, dtype=np.float32), np.ones((16, 16), np.float32))
    ident = np.eye(128, dtype=np.float16)
    in_maps = []
    for c in range(N_CORES):
        ws = wf[:, c * NPC : (c + 1) * NPC]
        wtiled = ws.reshape(KC, 128, NPC).transpose(1, 0, 2)  # [128, KC, NPC]
        parts = [x2[:, 0, :]]
        for g in range(7):
            parts.append(x2[:, g + 1, :])
            parts.append(wtiled[:, 4 * g : 4 * g + 4, :].reshape(128, 4 * NPC))
        parts.append(wtiled[:, 28:32, :].reshape(128, 4 * NPC))
        wxm = np.ascontiguousarray(np.concatenate(parts, axis=1))
        in_maps.append(
            {
                "wx": wxm,
                "b": np.ascontiguousarray(bf[c * NPC : (c + 1) * NPC]).reshape(
                    1, NPC
                ),
                "gg": gg,
                "idin": ident,
            }
        )
    return in_maps


def _make_in_maps_v3(x, weight, bias):
    x2 = np.ascontiguousarray(
        np.asarray(x, dtype=np.float16)
        .reshape(M, K // 512, 512)
        .transpose(1, 0, 2)
    )
    wf = np.asarray(weight, dtype=np.float16)
    bf = np.asarray(bias, dtype=np.float16)
    gg = np.kron(np.eye(8, dtype=np.float32), np.ones((16, 16), np.float32))
    ident = np.eye(128, dtype=np.float16)
    in_maps = []
    for c in range(N_CORES):
        ws = wf[:, c * NPC : (c + 1) * NPC]
        wtiled = np.ascontiguousarray(
            ws.reshape(KC, 128, NPC).transpose(1, 0, 2).reshape(128, KC * NPC)
        )
        in_maps.append(
            {
                "x": x2,
                "wp": wtiled,
                "b": np.ascontiguousarray(bf[c * NPC : (c + 1) * NPC]).reshape(
                    1, NPC
                ),
                "gg": gg,
                "idin": ident,
            }
        )
    return in_maps


def _make_in_maps_v2(x, weight, bias, variant="v2"):
    fused_x = variant.startswith("v2f")
    xflat = np.ascontiguousarray(np.asarray(x, dtype=np.float16).reshape(M, K))
    wf = np.asarray(weight, dtype=np.float16)
    bf = np.asarray(bias, dtype=np.float16)
    gg = np.kron(np.eye(8, dtype=np.float32), np.ones((16, 16), np.float32))
    ident = np.eye(128, dtype=np.float16)
    in_maps = []
    for c in range(N_CORES):
        ws = wf[:, c * NPC : (c + 1) * NPC]
        # partition p holds row kc*128+p of every K-chunk, chunk-major
        wtiled = np.ascontiguousarray(
            ws.reshape(KC, 128, NPC).transpose(1, 0, 2).reshape(128, KC * NPC)
        )
        if fused_x:
            wtiled = np.ascontiguousarray(
                np.concatenate([xflat, wtiled], axis=1)
            )
        m = {
            "wp": wtiled,
            "b": np.ascontiguousarray(bf[c * NPC : (c + 1) * NPC]).reshape(1, NPC),
            "gg": gg,
            "idin": ident,
        }
        if not fused_x:
            m["xf"] = xflat
        in_maps.append(m)
    return in_maps


def _get_nc(bias_nonzero: bool, loop_reps: int = 1, variant: str = ""):
    key = ("nc", bias_nonzero, loop_reps, variant)
    if key not in _STATE:
        if variant.startswith("v6"):
            _STATE[key] = _build_v6(bias_nonzero, loop_reps, variant)
        elif variant.startswith("v4") or variant.startswith("v5"):
            _STATE[key] = _build_v4(bias_nonzero, loop_reps, variant)
        elif variant.startswith("v3"):
            _STATE[key] = _build_v3(bias_nonzero, loop_reps, variant)
        elif variant.startswith("v2"):
            _STATE[key] = _build_v2(bias_nonzero, loop_reps, variant)
        else:
            _STATE[key] = _build(bias_nonzero, loop_reps, variant)
    return _STATE[key]


def _make_in_maps(x, weight, bias):
    x2 = np.ascontiguousarray(
        np.asarray(x, dtype=np.float16)
        .reshape(M, K // 512, 512)
        .transpose(1, 0, 2)
    )
    wf = np.asarray(weight, dtype=np.float16)
    bf = np.asarray(bias, dtype=np.float16)
    gg = np.kron(np.eye(8, dtype=np.float32), np.ones((16, 16), np.float32))
    ident = np.eye(128, dtype=np.float16)
    in_maps = []
    for c in range(N_CORES):
        in_maps.append(
            {
                "x": x2,
                "w": np.ascontiguousarray(wf[:, c * NPC : (c + 1) * NPC]),
                "b": np.ascontiguousarray(bf[c * NPC : (c + 1) * NPC]).reshape(
                    1, NPC
                ),
                "gg": gg,
                "idin": ident,
            }
        )
    return in_maps


DEFAULT_VARIANT = "v6f"


def kernel(x, weight, bias, _trace=False):
    from concourse.bass_utils import run_bass_kernel_spmd

    bias_nonzero = bool(np.any(np.asarray(bias)))
    nc = _get_nc(bias_nonzero, variant=DEFAULT_VARIANT)
    in_maps = _make_in_maps_v6(x, weight, bias)
    res = run_bass_kernel_spmd(
        nc, in_maps, core_ids=list(range(N_CORES)), trace=_trace
    )
    _STATE["last_results"] = res
    y = np.concatenate([res.results[c]["y"] for c in range(N_CORES)], axis=1)
    return y.reshape(M, 1, N_FULL).astype(np.float16)



# revision 32
# speedup vs baseline: 1.0080x; 1.0080x over previous
"""BlockSparseThresLinear Trainium2 kernel.

Problem (hardcoded): x (128,1,4096) f16, weight (4096,11008) f16, bias (11008,) f16.
  BLOCK_M=16, BLOCK_K=64, THRES=0.8: per (16,64) block of x.reshape(128,4096),
  mask = mean(|block|, fp32) > 0.8; y = (x * mask_expanded) @ weight + bias.

Sharding: weight/bias column-sharded across 8 cores (1376 cols each); x
replicated; each core computes its output slice independently; host concats.

Shipping pipeline = _build_v6, variant "v6f" (HW ~41.1us/iter vs ~44.1 for
the legacy _build baseline; per-core DMA floor ~12.7MB at the measured
~334GB/s plateau ≈ 38us):
  - ONE packed DRAM tensor per core, host-pretiled so every DMA reads a
    fully contiguous per-partition run with ~12KB descriptors (HW plateau;
    32 separate 352KB chunk DMAs measured 36.2us for W alone vs 34.8us for
    4-chunk groups): [BB-as-f16-bytes | xT chunks 0-3 | 6 x (xT(4g+4..) |
    W chunks 4g..4g+3) | xT(28-31) | W24 | W25..29 | W30 | W31].
  - x ships PRE-TRANSPOSED (xT chunk = [128 k-part, 128 m]) inside the W
    group DMAs, one group ahead of its consumers - no separate x DMAs to
    lose DMA-pool arbitration to the W stream, and NO on-device transposes
    (the old PE-transpose->PSUM->DVE-copy chain serialized against matmuls
    via each engine's in-order queue).
  - masks computed in transposed space per 4-chunk group: DVE abs-reduce
    over 16-wide m-groups -> pbs [128,32] f32; one PE matmul with
    BB = kron(eye(2), ones(64)) (f32, shipped as raw bytes in the f16
    stream, bitcast back) sums each k-block's 64 partitions; DVE is_gt
    819.2 -> maskT; DVE mult (broadcast AP) -> xmT = matmul lhsT directly.
  - W stream: head DMA (BB+xT0) then 6 fused groups on the sync/HWDGE
    queue back-to-back; tail fine-grained ([xT28-31|W24], W25..29 singles,
    W30/31 in 3 psum-slice pieces) so the last W bytes feed a short
    mm->copy->y chain; 3 PE matmuls per chunk accumulate per-slice PSUM
    tiles [128,{512,512,352}].
  - y emits slice-major as each slice's stop-matmul lands: DVE psum->f16
    copy, then DMA (middle slice on the scalar queue, outers on sync).
  - loop_reps>1 wraps the body in tc.For_i for differential benchmarking.
"""

import numpy as np

M = 128
K = 4096
N_FULL = 11008
N_CORES = 8
NPC = N_FULL // N_CORES  # 1376
KC = K // 128  # 32 chunks
THRES_SUM = 819.2  # 0.8 * 1024 (exact in fp32: matches (sum/1024) > 0.8f)

_STATE = {}


def _build(bias_nonzero: bool, loop_reps: int = 1, variant: str = ""):
    from contextlib import ExitStack

    import concourse.bacc as bacc
    import concourse.bass as bass
    import concourse.mybir as mybir
    import concourse.tile as tile

    f16 = mybir.dt.float16
    f32 = mybir.dt.float32

    nc = bacc.Bacc(
        "TRN2",
        target_bir_lowering=False,
        debug=False,
        enable_asserts=False,
        num_devices=N_CORES,
    )

    if variant == "xstrided":
        x = nc.dram_tensor("x", [M, K], f16, kind="ExternalInput").ap()
    else:
        # chunk-major x layout (host repacks): each x chunk DMA reads a
        # contiguous 128KB region -- HW-measured ~2us faster than strided
        x = nc.dram_tensor("x", [K // 512, M, 512], f16, kind="ExternalInput").ap()
    w = nc.dram_tensor("w", [K, NPC], f16, kind="ExternalInput").ap()
    b = nc.dram_tensor("b", [1, NPC], f16, kind="ExternalInput").ap()
    gg = nc.dram_tensor("gg", [M, M], f32, kind="ExternalInput").ap()
    idin = nc.dram_tensor("idin", [128, 128], f16, kind="ExternalInput").ap()
    if variant == "tp":
        # three contiguous output tensors (host concatenates) -- avoids
        # strided DRAM writes on the critical tail
        youts = [
            nc.dram_tensor(f"y{i}", [M, wd], f16, kind="ExternalOutput").ap()
            for i, wd in enumerate((512, 512, NPC - 1024))
        ]
    else:
        y = nc.dram_tensor("y", [M, NPC], f16, kind="ExternalOutput").ap()

    # Output N split into PSUM-bank-sized slices (<=512 fp32 per bank).
    n_slices = [(0, 512), (512, 1024), (1024, NPC)]

    XCH = 8
    xw = K // XCH  # 512 cols = 8 blocks = 4 K-chunks per x chunk
    KC_G = KC // XCH

    with tile.TileContext(nc) as tc, ExitStack() as ctx:
        if loop_reps > 1:
            # benchmark-only: repeat the whole pipeline on-device so
            # differential wall timing can resolve the per-iteration time
            ctx.enter_context(tc.For_i(0, loop_reps, 1))
        singles = ctx.enter_context(tc.tile_pool(name="singles", bufs=1))
        wbufs = {"wb8": 8, "wb12": 12, "wb28": 28}.get(variant, 20)
        wpool = ctx.enter_context(tc.tile_pool(name="wpool", bufs=wbufs))
        xmpool = ctx.enter_context(tc.tile_pool(name="xmpool", bufs=8))
        xmtpool = ctx.enter_context(tc.tile_pool(name="xmtpool", bufs=KC))
        mrpool = ctx.enter_context(tc.tile_pool(name="mrpool", bufs=4))
        outpool = ctx.enter_context(tc.tile_pool(name="outpool", bufs=1))
        wlpool = ctx.enter_context(tc.tile_pool(name="wlpool", bufs=2))
        ps_t = ctx.enter_context(tc.tile_pool(name="ps_t", bufs=3, space="PSUM"))
        ps_y = ctx.enter_context(tc.tile_pool(name="ps_y", bufs=1, space="PSUM"))
        ps_m = ctx.enter_context(tc.tile_pool(name="ps_m", bufs=2, space="PSUM"))

        # Prologue DMAs split across independent dispatch resources: x0/gg/
        # ident on the scalar/HWDGE queue, bulk x chunks on gpsimd/SWDGE
        # (parallel dispatcher), so the sync/HWDGE queue carries nothing but
        # the W stream.
        xtiles = []
        for c in range(XCH):
            xsb = singles.tile([M, xw], f16, tag=f"xsb{c}")
            eng = nc.scalar if c == 0 else nc.gpsimd
            xin = x[:, c * xw : (c + 1) * xw] if variant == "xstrided" else x[c]
            eng.dma_start(out=xsb[:], in_=xin)
            xtiles.append(xsb)

        ggs = singles.tile([M, M], f32)
        nc.scalar.dma_start(out=ggs[:], in_=gg[:])
        ident = singles.tile([128, 128], f16)
        nc.scalar.dma_start(out=ident[:], in_=idin[:])

        if bias_nonzero:
            bias_b = singles.tile([M, NPC], f16)
            bcast = bass.AP(tensor=b.tensor, offset=b.offset, ap=[[0, M], b.ap[1]])
            nc.sync.dma_start(out=bias_b[:], in_=bcast)

        ypsums = {}
        for i, (lo, hi) in enumerate(n_slices):
            yps_tile = ps_y.tile([M, hi - lo], f32, tag=f"ypsum{i}")
            ypsums[lo] = yps_tile
        ysb = outpool.tile([M, NPC], f16)

        def emit_out_range(pk, a, bnd):
            # PSUM[pk] sub-range -> f16 SBUF (+bias) on DVE, then DMA out.
            # DVE-only keeps ACT a pure DMA-dispatch queue (no LoadActFuncSet
            # table load blocking the x0 dispatch).
            if bias_nonzero:
                nc.vector.tensor_tensor(
                    out=ysb[:, a:bnd],
                    in0=ypsums[pk][:, a - pk : bnd - pk],
                    in1=bias_b[:, a:bnd],
                    op=mybir.AluOpType.add,
                )
            else:
                nc.vector.tensor_copy(
                    out=ysb[:, a:bnd], in_=ypsums[pk][:, a - pk : bnd - pk]
                )
            # middle slice on the scalar queue so y dispatches overlap
            eng = nc.scalar if a == 512 else nc.sync
            if variant == "tp":
                eng.dma_start(out=youts[[0, 512, 1024].index(pk)][:, a - pk :], in_=ysb[:, a:bnd])
            else:
                eng.dma_start(out=y[:, a:bnd], in_=ysb[:, a:bnd])

        xmt_tail = {}
        for c in range(XCH):
            xsb = xtiles[c]
            nbl = xw // 64  # 16 blocks
            bsum = mrpool.tile([M, nbl], f32, tag="bsum")
            nc.vector.tensor_reduce(
                out=bsum[:],
                in_=xsb[:].rearrange("p (b q) -> p b q", q=64),
                axis=mybir.AxisListType.X,
                op=mybir.AluOpType.add,
                apply_absolute_value=True,
            )
            gsum = ps_m.tile([M, nbl], f32)
            nc.tensor.matmul(gsum[:], lhsT=ggs[:], rhs=bsum[:], start=True, stop=True)
            maskrow = mrpool.tile([M, nbl], f16, tag="maskrow")
            nc.vector.tensor_scalar(
                out=maskrow[:],
                in0=gsum[:],
                scalar1=float(THRES_SUM),
                scalar2=None,
                op0=mybir.AluOpType.is_gt,
            )

            wsb2 = None
            for j in range(KC_G):
                kc = c * KC_G + j
                tailk = kc >= KC - 2
                if not tailk and variant == "wpair":
                    # one DMA per K-chunk PAIR (704KB) halves W DMA count
                    if j % 2 == 0:
                        wsb2 = wpool.tile([128, 2, NPC], f16, tag="wsb2")
                        nc.sync.dma_start(
                            out=wsb2[:],
                            in_=w[kc * 128 : (kc + 2) * 128, :].rearrange(
                                "(a p) n -> p a n", p=128
                            ),
                        )
                    wsb = wsb2[:, j % 2, :]
                elif not tailk:
                    wsb_t = wpool.tile([128, NPC], f16, tag="wsb")
                    weng = nc.scalar if (variant == "w2q" and kc % 2) else nc.sync
                    weng.dma_start(
                        out=wsb_t[:], in_=w[kc * 128 : (kc + 1) * 128, :]
                    )
                    wsb = wsb_t[:]

                xm = xmpool.tile([128, 128], f16)
                mview = maskrow[:, 2 * j : 2 * j + 2].unsqueeze(2).broadcast_to(
                    [128, 2, 64]
                )
                nc.vector.tensor_tensor(
                    out=xm[:].rearrange("p (b q) -> p b q", q=64),
                    in0=xsb[:, j * 128 : (j + 1) * 128].rearrange(
                        "p (b q) -> p b q", q=64
                    ),
                    in1=mview,
                    op=mybir.AluOpType.mult,
                )

                pst = ps_t.tile([128, 128], f16)
                nc.tensor.transpose(pst[:], xm[:], ident[:])
                xmt = xmtpool.tile([128, 128], f16)
                nc.vector.tensor_copy(out=xmt[:], in_=pst[:])

                if not tailk:
                    for lo, hi in n_slices:
                        nc.tensor.matmul(
                            ypsums[lo][:],
                            lhsT=xmt[:],
                            rhs=wsb[:, lo:hi],
                            start=(kc == 0),
                            stop=False,
                        )
                else:
                    xmt_tail[kc] = xmt
                    if kc == KC - 1:
                        tail_pieces = [
                            (0, 0, 512),
                            (512, 512, 1024),
                            (1024, 1024, NPC),
                        ]
                        if variant == "tp":
                            # full contiguous tail-chunk DMAs (no strided
                            # piece reads); per-slice gemm/copy/out pipeline
                            wtl = {}
                            for kk in (KC - 2, KC - 1):
                                wt = wlpool.tile(
                                    [128, NPC], f16, tag=f"wt{kk % 2}"
                                )
                                nc.sync.dma_start(
                                    out=wt[:],
                                    in_=w[kk * 128 : (kk + 1) * 128, :],
                                )
                                wtl[kk] = wt
                            for pk, a, bnd in tail_pieces:
                                for kk in (KC - 2, KC - 1):
                                    nc.tensor.matmul(
                                        ypsums[pk][:, a - pk : bnd - pk],
                                        lhsT=xmt_tail[kk][:],
                                        rhs=wtl[kk][:, a:bnd],
                                        start=False,
                                        stop=(kk == KC - 1),
                                    )
                                emit_out_range(pk, a, bnd)
                        else:
                            # Final two K-chunks stream slice-major: each
                            # slice's last gemms -> psum copy -> output DMA
                            # pipeline while later slices still stream.
                            for pk, a, bnd in tail_pieces:
                                for kk in (KC - 2, KC - 1):
                                    wl = wlpool.tile(
                                        [128, bnd - a], f16, tag=f"wl{a}_{kk % 2}"
                                    )
                                    wleng = (
                                        nc.scalar
                                        if (variant == "w2q" and kk % 2)
                                        else nc.sync
                                    )
                                    wleng.dma_start(
                                        out=wl[:],
                                        in_=w[kk * 128 : (kk + 1) * 128, a:bnd],
                                    )
                                    nc.tensor.matmul(
                                        ypsums[pk][:, a - pk : bnd - pk],
                                        lhsT=xmt_tail[kk][:],
                                        rhs=wl[:],
                                        start=False,
                                        stop=(kk == KC - 1),
                                    )
                                emit_out_range(pk, a, bnd)

    nc.compile()
    return nc


def _build_v2(bias_nonzero: bool, loop_reps: int = 1, variant: str = "v2"):
    """Group-DMA pipeline: W host-pretiled so partition p holds row kc*128+p
    of every K-chunk; the stream is 7 DMAs of 4 K-chunks (desc 11008B, HW
    plateau ~34.8us for the 11.27MB) + a fine-grained tail (2+1+3-piece) so
    the last W bytes feed a short matmul->copy->y chain. x loads as ONE flat
    [128,4096] DMA ("v2": scalar queue; "v2f": fused into W group 0). One
    DVE reduce computes all 64 block sums; one PE matmul + is_gt gives the
    full [128,64] mask. y emits per-slice on the scalar queue ("...1": one
    [128,1376] DMA at the end instead).
    """
    from contextlib import ExitStack

    import concourse.bacc as bacc
    import concourse.bass as bass
    import concourse.mybir as mybir
    import concourse.tile as tile

    f16 = mybir.dt.float16
    f32 = mybir.dt.float32

    fused_x = variant.startswith("v2f")
    one_y = "1" in variant[2:]
    wbufs = 7 if "b7" in variant else 4

    nc = bacc.Bacc(
        "TRN2",
        target_bir_lowering=False,
        debug=False,
        enable_asserts=False,
        num_devices=N_CORES,
    )

    GS = 4  # K-chunks per W group DMA
    NG = KC // GS  # 8 groups; last group streams fine-grained
    WG = GS * NPC  # 5504 cols per full group
    wp_cols = KC * NPC + (K if fused_x else 0)
    wp = nc.dram_tensor("wp", [128, wp_cols], f16, kind="ExternalInput").ap()
    if not fused_x:
        xf = nc.dram_tensor("xf", [M, K], f16, kind="ExternalInput").ap()
    b = nc.dram_tensor("b", [1, NPC], f16, kind="ExternalInput").ap()
    gg = nc.dram_tensor("gg", [M, M], f32, kind="ExternalInput").ap()
    idin = nc.dram_tensor("idin", [128, 128], f16, kind="ExternalInput").ap()
    y = nc.dram_tensor("y", [M, NPC], f16, kind="ExternalOutput").ap()

    n_slices = [(0, 512), (512, 1024), (1024, NPC)]
    # offset of chunk kc's W columns inside the packed wp row
    xoff = K if fused_x else 0

    def wcol(kc, c0=0):
        return xoff + kc * NPC + c0

    with tile.TileContext(nc) as tc, ExitStack() as ctx:
        if loop_reps > 1:
            ctx.enter_context(tc.For_i(0, loop_reps, 1))
        singles = ctx.enter_context(tc.tile_pool(name="singles", bufs=1))
        xpool = ctx.enter_context(tc.tile_pool(name="xpool", bufs=2))
        wpool = ctx.enter_context(tc.tile_pool(name="wpool", bufs=wbufs))
        tlpool = ctx.enter_context(tc.tile_pool(name="tlpool", bufs=2))
        xmpool = ctx.enter_context(tc.tile_pool(name="xmpool", bufs=8))
        xmtpool = ctx.enter_context(tc.tile_pool(name="xmtpool", bufs=KC))
        mrpool = ctx.enter_context(tc.tile_pool(name="mrpool", bufs=2))
        outpool = ctx.enter_context(tc.tile_pool(name="outpool", bufs=2))
        ps_t = ctx.enter_context(tc.tile_pool(name="ps_t", bufs=3, space="PSUM"))
        ps_y = ctx.enter_context(tc.tile_pool(name="ps_y", bufs=1, space="PSUM"))
        ps_m = ctx.enter_context(tc.tile_pool(name="ps_m", bufs=2, space="PSUM"))

        # head loads on the scalar/ACT HWDGE queue; W owns the sync/SP queue
        ggs = singles.tile([M, M], f32)
        nc.scalar.dma_start(out=ggs[:], in_=gg[:])
        ident = singles.tile([128, 128], f16)
        nc.scalar.dma_start(out=ident[:], in_=idin[:])
        if bias_nonzero:
            bias_b = singles.tile([M, NPC], f16)
            bcast = bass.AP(tensor=b.tensor, offset=b.offset, ap=[[0, M], b.ap[1]])
            nc.scalar.dma_start(out=bias_b[:], in_=bcast)

        # W group DMAs: groups 0..6 coarse; group 7 = 2-chunk + 1-chunk +
        # three slice pieces of the final chunk (tail pipelining)
        wtiles = {}
        if fused_x:
            g0 = xpool.tile([128, K + WG], f16, tag="g0")
            nc.sync.dma_start(out=g0[:], in_=wp[:, : K + WG])
            xsb = g0[:, :K]
            wtiles[0] = g0
        else:
            xsb_t = xpool.tile([M, K], f16, tag="xsb")
            nc.scalar.dma_start(out=xsb_t[:], in_=xf[:])
            xsb = xsb_t[:]
            w0 = wpool.tile([128, WG], f16, tag="wg")
            nc.sync.dma_start(out=w0[:], in_=wp[:, xoff : xoff + WG])
            wtiles[0] = w0
        for g in range(1, NG - 1):
            wg = wpool.tile([128, WG], f16, tag="wg")
            nc.sync.dma_start(
                out=wg[:], in_=wp[:, wcol(g * GS) : wcol((g + 1) * GS)]
            )
            wtiles[g] = wg
        w2829 = tlpool.tile([128, 2 * NPC], f16, tag="w2829")
        nc.sync.dma_start(out=w2829[:], in_=wp[:, wcol(28) : wcol(30)])
        w30 = tlpool.tile([128, NPC], f16, tag="w30")
        nc.sync.dma_start(out=w30[:], in_=wp[:, wcol(30) : wcol(31)])
        wl31 = {}
        for lo, hi in n_slices:
            wl = tlpool.tile([128, hi - lo], f16, tag=f"wl31_{lo}")
            nc.sync.dma_start(out=wl[:], in_=wp[:, wcol(31, lo) : wcol(31, hi)])
            wl31[lo] = wl

        # mask: one reduce over all 64 blocks, one PE group-sum, one is_gt
        bsum = mrpool.tile([M, K // 64], f32, tag="bsum")
        nc.vector.tensor_reduce(
            out=bsum[:],
            in_=xsb.rearrange("p (b q) -> p b q", q=64),
            axis=mybir.AxisListType.X,
            op=mybir.AluOpType.add,
            apply_absolute_value=True,
        )
        gsum = ps_m.tile([M, K // 64], f32)
        nc.tensor.matmul(gsum[:], lhsT=ggs[:], rhs=bsum[:], start=True, stop=True)
        maskrow = mrpool.tile([M, K // 64], f16, tag="maskrow")
        nc.vector.tensor_scalar(
            out=maskrow[:],
            in0=gsum[:],
            scalar1=float(THRES_SUM),
            scalar2=None,
            op0=mybir.AluOpType.is_gt,
        )

        ypsums = {}
        for i, (lo, hi) in enumerate(n_slices):
            yps_tile = ps_y.tile([M, hi - lo], f32, tag=f"ypsum{i}")
            ypsums[lo] = yps_tile
        ysb = outpool.tile([M, NPC], f16)

        def make_xmt(kc):
            xm = xmpool.tile([128, 128], f16)
            mview = maskrow[:, 2 * kc : 2 * kc + 2].unsqueeze(2).broadcast_to(
                [128, 2, 64]
            )
            nc.vector.tensor_tensor(
                out=xm[:].rearrange("p (b q) -> p b q", q=64),
                in0=xsb[:, kc * 128 : (kc + 1) * 128].rearrange(
                    "p (b q) -> p b q", q=64
                ),
                in1=mview,
                op=mybir.AluOpType.mult,
            )
            pst = ps_t.tile([128, 128], f16)
            nc.tensor.transpose(pst[:], xm[:], ident[:])
            xmt = xmtpool.tile([128, 128], f16)
            nc.vector.tensor_copy(out=xmt[:], in_=pst[:])
            return xmt

        def emit_y(pk, a, bnd):
            if bias_nonzero:
                nc.vector.tensor_tensor(
                    out=ysb[:, a:bnd],
                    in0=ypsums[pk][:, a - pk : bnd - pk],
                    in1=bias_b[:, a:bnd],
                    op=mybir.AluOpType.add,
                )
            else:
                nc.vector.tensor_copy(
                    out=ysb[:, a:bnd], in_=ypsums[pk][:, a - pk : bnd - pk]
                )
            if not one_y:
                nc.scalar.dma_start(out=y[:, a:bnd], in_=ysb[:, a:bnd])

        xmts = {}
        for kc in range(KC):
            xmts[kc] = make_xmt(kc)
            if kc < 28:
                g, j = kc // GS, kc % GS
                wv = wtiles[g][:, (xoff if fused_x and g == 0 else 0) :]
                for lo, hi in n_slices:
                    nc.tensor.matmul(
                        ypsums[lo][:],
                        lhsT=xmts[kc][:],
                        rhs=wv[:, j * NPC + lo : j * NPC + hi],
                        start=(kc == 0),
                        stop=False,
                    )
            elif kc in (28, 29):
                for lo, hi in n_slices:
                    nc.tensor.matmul(
                        ypsums[lo][:],
                        lhsT=xmts[kc][:],
                        rhs=w2829[:, (kc - 28) * NPC + lo : (kc - 28) * NPC + hi],
                        start=False,
                        stop=False,
                    )
            elif kc == 30:
                for lo, hi in n_slices:
                    nc.tensor.matmul(
                        ypsums[lo][:],
                        lhsT=xmts[kc][:],
                        rhs=w30[:, lo:hi],
                        start=False,
                        stop=False,
                    )
            else:
                for lo, hi in n_slices:
                    nc.tensor.matmul(
                        ypsums[lo][:],
                        lhsT=xmts[kc][:],
                        rhs=wl31[lo][:],
                        start=False,
                        stop=True,
                    )
                    emit_y(lo, lo, hi)
        if one_y:
            nc.scalar.dma_start(out=y[:], in_=ysb[:])

    nc.compile()
    return nc


def _build_v3(bias_nonzero: bool, loop_reps: int = 1, variant: str = "v3"):
    """Baseline's fine-grained per-chunk compute pipeline (x in 8 chunk DMAs,
    per-chunk mask chain, deep xmt pool) with the W stream restructured into
    pretiled 4-K-chunk group DMAs (desc 11008B — HW plateau ~34.8us vs 36.2us
    for 32 single-chunk DMAs). Tail: chunks 28-29 single-chunk DMAs, chunks
    30-31 slice-major pieces with per-slice y emission.

    variant flags after "v3": 'y' = all y DMAs on scalar queue (default
    baseline mix: s1 scalar, s0/s2 sync).
    """
    from contextlib import ExitStack

    import concourse.bacc as bacc
    import concourse.bass as bass
    import concourse.mybir as mybir
    import concourse.tile as tile

    f16 = mybir.dt.float16
    f32 = mybir.dt.float32

    y_scalar = "y" in variant[2:]

    nc = bacc.Bacc(
        "TRN2",
        target_bir_lowering=False,
        debug=False,
        enable_asserts=False,
        num_devices=N_CORES,
    )

    GS = 4
    WG = GS * NPC
    x = nc.dram_tensor("x", [K // 512, M, 512], f16, kind="ExternalInput").ap()
    wp = nc.dram_tensor("wp", [128, KC * NPC], f16, kind="ExternalInput").ap()
    b = nc.dram_tensor("b", [1, NPC], f16, kind="ExternalInput").ap()
    gg = nc.dram_tensor("gg", [M, M], f32, kind="ExternalInput").ap()
    idin = nc.dram_tensor("idin", [128, 128], f16, kind="ExternalInput").ap()
    y = nc.dram_tensor("y", [M, NPC], f16, kind="ExternalOutput").ap()

    n_slices = [(0, 512), (512, 1024), (1024, NPC)]

    def wcol(kc, c0=0):
        return kc * NPC + c0

    XCH = 8
    xw = K // XCH

    with tile.TileContext(nc) as tc, ExitStack() as ctx:
        if loop_reps > 1:
            ctx.enter_context(tc.For_i(0, loop_reps, 1))
        singles = ctx.enter_context(tc.tile_pool(name="singles", bufs=1))
        wpool = ctx.enter_context(tc.tile_pool(name="wpool", bufs=4))
        tlpool = ctx.enter_context(tc.tile_pool(name="tlpool", bufs=2))
        xmpool = ctx.enter_context(tc.tile_pool(name="xmpool", bufs=8))
        xmtpool = ctx.enter_context(tc.tile_pool(name="xmtpool", bufs=KC))
        mrpool = ctx.enter_context(tc.tile_pool(name="mrpool", bufs=4))
        outpool = ctx.enter_context(tc.tile_pool(name="outpool", bufs=1))
        ps_t = ctx.enter_context(tc.tile_pool(name="ps_t", bufs=3, space="PSUM"))
        ps_y = ctx.enter_context(tc.tile_pool(name="ps_y", bufs=1, space="PSUM"))
        ps_m = ctx.enter_context(tc.tile_pool(name="ps_m", bufs=2, space="PSUM"))

        # x chunks: first on scalar/HWDGE, rest on gpsimd/SWDGE (baseline)
        xtiles = []
        for c in range(XCH):
            xsb = singles.tile([M, xw], f16, tag=f"xsb{c}")
            eng = nc.scalar if c == 0 else nc.gpsimd
            eng.dma_start(out=xsb[:], in_=x[c])
            xtiles.append(xsb)

        ggs = singles.tile([M, M], f32)
        nc.scalar.dma_start(out=ggs[:], in_=gg[:])
        ident = singles.tile([128, 128], f16)
        nc.scalar.dma_start(out=ident[:], in_=idin[:])

        if bias_nonzero:
            bias_b = singles.tile([M, NPC], f16)
            bcast = bass.AP(tensor=b.tensor, offset=b.offset, ap=[[0, M], b.ap[1]])
            nc.scalar.dma_start(out=bias_b[:], in_=bcast)

        # W group DMAs for chunks 0..27 (7 groups of 4)
        wgroups = {}
        for g in range(7):
            wg_t = wpool.tile([128, WG], f16, tag="wg")
            nc.sync.dma_start(
                out=wg_t[:], in_=wp[:, wcol(g * GS) : wcol((g + 1) * GS)]
            )
            wgroups[g] = wg_t
        # tail: 28, 29 single chunks; 30-31 slice-major pieces
        wtail = {}
        for kk in (28, 29):
            wt = tlpool.tile([128, NPC], f16, tag=f"wt{kk}")
            nc.sync.dma_start(out=wt[:], in_=wp[:, wcol(kk) : wcol(kk + 1)])
            wtail[kk] = wt
        wl = {}
        for lo, hi in n_slices:
            for kk in (30, 31):
                wl_t = tlpool.tile([128, hi - lo], f16, tag=f"wl{lo}_{kk}")
                nc.sync.dma_start(
                    out=wl_t[:], in_=wp[:, wcol(kk, lo) : wcol(kk, hi)]
                )
                wl[(kk, lo)] = wl_t

        ypsums = {}
        for i, (lo, hi) in enumerate(n_slices):
            yps_tile = ps_y.tile([M, hi - lo], f32, tag=f"ypsum{i}")
            ypsums[lo] = yps_tile
        ysb = outpool.tile([M, NPC], f16)

        def emit_out_range(pk, a, bnd):
            if bias_nonzero:
                nc.vector.tensor_tensor(
                    out=ysb[:, a:bnd],
                    in0=ypsums[pk][:, a - pk : bnd - pk],
                    in1=bias_b[:, a:bnd],
                    op=mybir.AluOpType.add,
                )
            else:
                nc.vector.tensor_copy(
                    out=ysb[:, a:bnd], in_=ypsums[pk][:, a - pk : bnd - pk]
                )
            eng = nc.scalar if (y_scalar or a == 512) else nc.sync
            eng.dma_start(out=y[:, a:bnd], in_=ysb[:, a:bnd])

        xmt_all = {}
        for c in range(XCH):
            xsb = xtiles[c]
            nbl = xw // 64
            bsum = mrpool.tile([M, nbl], f32, tag="bsum")
            nc.vector.tensor_reduce(
                out=bsum[:],
                in_=xsb[:].rearrange("p (b q) -> p b q", q=64),
                axis=mybir.AxisListType.X,
                op=mybir.AluOpType.add,
                apply_absolute_value=True,
            )
            gsum = ps_m.tile([M, nbl], f32)
            nc.tensor.matmul(
                gsum[:], lhsT=ggs[:], rhs=bsum[:], start=True, stop=True
            )
            maskrow = mrpool.tile([M, nbl], f16, tag="maskrow")
            nc.vector.tensor_scalar(
                out=maskrow[:],
                in0=gsum[:],
                scalar1=float(THRES_SUM),
                scalar2=None,
                op0=mybir.AluOpType.is_gt,
            )

            for j in range(4):
                kc = c * 4 + j
                xm = xmpool.tile([128, 128], f16)
                mview = maskrow[:, 2 * j : 2 * j + 2].unsqueeze(2).broadcast_to(
                    [128, 2, 64]
                )
                nc.vector.tensor_tensor(
                    out=xm[:].rearrange("p (b q) -> p b q", q=64),
                    in0=xsb[:, j * 128 : (j + 1) * 128].rearrange(
                        "p (b q) -> p b q", q=64
                    ),
                    in1=mview,
                    op=mybir.AluOpType.mult,
                )
                pst = ps_t.tile([128, 128], f16)
                nc.tensor.transpose(pst[:], xm[:], ident[:])
                xmt = xmtpool.tile([128, 128], f16)
                nc.vector.tensor_copy(out=xmt[:], in_=pst[:])
                xmt_all[kc] = xmt

                if kc < 28:
                    wv = wgroups[kc // GS]
                    for lo, hi in n_slices:
                        nc.tensor.matmul(
                            ypsums[lo][:],
                            lhsT=xmt[:],
                            rhs=wv[:, (kc % GS) * NPC + lo : (kc % GS) * NPC + hi],
                            start=(kc == 0),
                            stop=False,
                        )
                elif kc in (28, 29):
                    for lo, hi in n_slices:
                        nc.tensor.matmul(
                            ypsums[lo][:],
                            lhsT=xmt[:],
                            rhs=wtail[kc][:, lo:hi],
                            start=False,
                            stop=False,
                        )
                elif kc == 31:
                    # slice-major: finish each slice then emit while later
                    # slices still stream
                    for lo, hi in n_slices:
                        for kk in (30, 31):
                            nc.tensor.matmul(
                                ypsums[lo][:],
                                lhsT=xmt_all[kk][:],
                                rhs=wl[(kk, lo)][:],
                                start=False,
                                stop=(kk == 31),
                            )
                        emit_out_range(lo, lo, hi)

    nc.compile()
    return nc


def _build_v4(bias_nonzero: bool, loop_reps: int = 1, variant: str = "v4"):
    """v3's fine-grained compute pipeline with x FUSED into the W stream:
    one packed DRAM tensor wx = [x0 | (x1|Wg0) | (x2|Wg1) | ... | (x7|Wg6) |
    W28..31]. Group g's single sync DMA (desc 12032B) delivers x chunk g+1
    one group ahead of its consumers; x0/gg/ident load tiny on scalar at the
    head. No separate x DMAs to lose DMA-pool arbitration to the W groups.

    flags after "v4": 'y' = all y DMAs on scalar (default: s1 scalar,
    s0/s2 sync). Variant "v5*": software-pipeline matmuls one group behind
    the mask/transpose chain so PE's in-order queue never stalls a
    transpose behind matmuls (breaks the per-chunk T->copy->mm latency
    round trip that capped the un-pipelined order at ~1.6us/chunk).
    """
    from contextlib import ExitStack

    import concourse.bacc as bacc
    import concourse.bass as bass
    import concourse.mybir as mybir
    import concourse.tile as tile

    f16 = mybir.dt.float16
    f32 = mybir.dt.float32

    y_scalar = "y" in variant[2:]
    pipelined = variant.startswith("v5")
    head_sync = "h" in variant[2:]

    nc = bacc.Bacc(
        "TRN2",
        target_bir_lowering=False,
        debug=False,
        enable_asserts=False,
        num_devices=N_CORES,
    )

    GS = 4
    WG = GS * NPC  # 5504
    GW = 512 + WG  # 6016 cols per fused group
    wx = nc.dram_tensor(
        "wx", [128, K + KC * NPC], f16, kind="ExternalInput"
    ).ap()
    b = nc.dram_tensor("b", [1, NPC], f16, kind="ExternalInput").ap()
    gg = nc.dram_tensor("gg", [M, M], f32, kind="ExternalInput").ap()
    idin = nc.dram_tensor("idin", [128, 128], f16, kind="ExternalInput").ap()
    y = nc.dram_tensor("y", [M, NPC], f16, kind="ExternalOutput").ap()

    n_slices = [(0, 512), (512, 1024), (1024, NPC)]
    TAIL0 = 512 + 7 * GW  # col offset of chunk 28

    def tailcol(kk, c0=0):
        return TAIL0 + (kk - 28) * NPC + c0

    with tile.TileContext(nc) as tc, ExitStack() as ctx:
        if loop_reps > 1:
            ctx.enter_context(tc.For_i(0, loop_reps, 1))
        singles = ctx.enter_context(tc.tile_pool(name="singles", bufs=1))
        x0pool = ctx.enter_context(tc.tile_pool(name="x0pool", bufs=2))
        wpool = ctx.enter_context(tc.tile_pool(name="wpool", bufs=8))
        tlpool = ctx.enter_context(tc.tile_pool(name="tlpool", bufs=2))
        xmpool = ctx.enter_context(tc.tile_pool(name="xmpool", bufs=8))
        xmtpool = ctx.enter_context(tc.tile_pool(name="xmtpool", bufs=KC))
        mrpool = ctx.enter_context(tc.tile_pool(name="mrpool", bufs=4))
        outpool = ctx.enter_context(tc.tile_pool(name="outpool", bufs=2))
        ps_t = ctx.enter_context(tc.tile_pool(name="ps_t", bufs=3, space="PSUM"))
        ps_y = ctx.enter_context(tc.tile_pool(name="ps_y", bufs=1, space="PSUM"))
        ps_m = ctx.enter_context(tc.tile_pool(name="ps_m", bufs=2, space="PSUM"))

        head_eng = nc.sync if head_sync else nc.scalar
        ggs = singles.tile([M, M], f32)
        head_eng.dma_start(out=ggs[:], in_=gg[:])
        ident = singles.tile([128, 128], f16)
        head_eng.dma_start(out=ident[:], in_=idin[:])
        x0 = x0pool.tile([M, 512], f16, tag="x0")
        head_eng.dma_start(out=x0[:], in_=wx[:, :512])

        if bias_nonzero:
            bias_b = singles.tile([M, NPC], f16)
            bcast = bass.AP(tensor=b.tensor, offset=b.offset, ap=[[0, M], b.ap[1]])
            nc.scalar.dma_start(out=bias_b[:], in_=bcast)

        # fused group DMAs: [x chunk g+1 | W chunks 4g..4g+3]
        wgroups = {}
        for g in range(7):
            wg_t = wpool.tile([128, GW], f16, tag="wg")
            nc.sync.dma_start(
                out=wg_t[:], in_=wx[:, 512 + g * GW : 512 + (g + 1) * GW]
            )
            wgroups[g] = wg_t
        wtail = {}
        for kk in (28, 29):
            wt = tlpool.tile([128, NPC], f16, tag=f"wt{kk}")
            nc.sync.dma_start(out=wt[:], in_=wx[:, tailcol(kk) : tailcol(kk + 1)])
            wtail[kk] = wt
        wl = {}
        for lo, hi in n_slices:
            for kk in (30, 31):
                wl_t = tlpool.tile([128, hi - lo], f16, tag=f"wl{lo}_{kk}")
                nc.sync.dma_start(
                    out=wl_t[:], in_=wx[:, tailcol(kk, lo) : tailcol(kk, hi)]
                )
                wl[(kk, lo)] = wl_t

        xtiles = [x0[:]] + [wgroups[g][:, :512] for g in range(7)]

        ypsums = {}
        for i, (lo, hi) in enumerate(n_slices):
            yps_tile = ps_y.tile([M, hi - lo], f32, tag=f"ypsum{i}")
            ypsums[lo] = yps_tile
        ysb = outpool.tile([M, NPC], f16)

        def emit_out_range(pk, a, bnd):
            if bias_nonzero:
                nc.vector.tensor_tensor(
                    out=ysb[:, a:bnd],
                    in0=ypsums[pk][:, a - pk : bnd - pk],
                    in1=bias_b[:, a:bnd],
                    op=mybir.AluOpType.add,
                )
            else:
                nc.vector.tensor_copy(
                    out=ysb[:, a:bnd], in_=ypsums[pk][:, a - pk : bnd - pk]
                )
            eng = nc.scalar if (y_scalar or a == 512) else nc.sync
            eng.dma_start(out=y[:, a:bnd], in_=ysb[:, a:bnd])

        xmt_all = {}

        def mask_and_transpose(c):
            xsb = xtiles[c]
            nbl = 8
            bsum = mrpool.tile([M, nbl], f32, tag="bsum")
            nc.vector.tensor_reduce(
                out=bsum[:],
                in_=xsb.rearrange("p (b q) -> p b q", q=64),
                axis=mybir.AxisListType.X,
                op=mybir.AluOpType.add,
                apply_absolute_value=True,
            )
            gsum = ps_m.tile([M, nbl], f32)
            nc.tensor.matmul(
                gsum[:], lhsT=ggs[:], rhs=bsum[:], start=True, stop=True
            )
            maskrow = mrpool.tile([M, nbl], f16, tag="maskrow")
            nc.vector.tensor_scalar(
                out=maskrow[:],
                in0=gsum[:],
                scalar1=float(THRES_SUM),
                scalar2=None,
                op0=mybir.AluOpType.is_gt,
            )
            for j in range(4):
                kc = c * 4 + j
                xm = xmpool.tile([128, 128], f16)
                mview = maskrow[:, 2 * j : 2 * j + 2].unsqueeze(2).broadcast_to(
                    [128, 2, 64]
                )
                nc.vector.tensor_tensor(
                    out=xm[:].rearrange("p (b q) -> p b q", q=64),
                    in0=xsb[:, j * 128 : (j + 1) * 128].rearrange(
                        "p (b q) -> p b q", q=64
                    ),
                    in1=mview,
                    op=mybir.AluOpType.mult,
                )
                pst = ps_t.tile([128, 128], f16)
                nc.tensor.transpose(pst[:], xm[:], ident[:])
                xmt = xmtpool.tile([128, 128], f16)
                nc.vector.tensor_copy(out=xmt[:], in_=pst[:])
                xmt_all[kc] = xmt

        def matmuls_for(kc):
            if kc < 28:
                wv = wgroups[kc // GS]
                for lo, hi in n_slices:
                    nc.tensor.matmul(
                        ypsums[lo][:],
                        lhsT=xmt_all[kc][:],
                        rhs=wv[
                            :,
                            512 + (kc % GS) * NPC + lo : 512
                            + (kc % GS) * NPC
                            + hi,
                        ],
                        start=(kc == 0),
                        stop=False,
                    )
            elif kc in (28, 29):
                for lo, hi in n_slices:
                    nc.tensor.matmul(
                        ypsums[lo][:],
                        lhsT=xmt_all[kc][:],
                        rhs=wtail[kc][:, lo:hi],
                        start=False,
                        stop=False,
                    )
            elif kc == 31:
                for lo, hi in n_slices:
                    for kk in (30, 31):
                        nc.tensor.matmul(
                            ypsums[lo][:],
                            lhsT=xmt_all[kk][:],
                            rhs=wl[(kk, lo)][:],
                            start=False,
                            stop=(kk == 31),
                        )
                    emit_out_range(lo, lo, hi)

        if pipelined:
            # group c's mask/transposes precede group c-1's matmuls in PE
            # program order, so a transpose never queues behind matmuls
            # whose W group hasn't streamed in yet
            for c in range(8):
                mask_and_transpose(c)
                if c >= 1:
                    for kc in range(4 * (c - 1), 4 * c):
                        matmuls_for(kc)
            for kc in range(28, 32):
                matmuls_for(kc)
        else:
            for c in range(8):
                mask_and_transpose(c)
                for kc in range(4 * c, 4 * c + 4):
                    matmuls_for(kc)

    nc.compile()
    return nc


def _build_v6(bias_nonzero: bool, loop_reps: int = 1, variant: str = "v6"):
    """Transpose-free pipeline: host supplies x PRE-TRANSPOSED (xT chunks of
    [128 k-part, 128 m]) packed into the fused W stream, so matmul lhsT comes
    straight from a DVE mask-multiply — no PE transposes, no PSUM round trip,
    no per-chunk copies. Masks are computed in transposed space per group:
    DVE reduce |xT| over 16-wide m-groups -> pbs [128, 4*8]; one PE matmul
    with BB = kron(eye(2), ones(64)) sums each k-block's 64 partitions ->
    mask_pre; is_gt -> maskT; DVE mult applies it. Stream layout per
    partition: [BBrow? no - BB separate f32 | xT0(chunks 0-3) | (xT(4g+4..7)
    | Wg) x7 | Wtail], head (BB, xT0) at the front of the sync queue.

    flags after "v6": 'y' = all y on scalar.
    """
    from contextlib import ExitStack

    import concourse.bacc as bacc
    import concourse.bass as bass
    import concourse.mybir as mybir
    import concourse.tile as tile

    f16 = mybir.dt.float16
    f32 = mybir.dt.float32

    y_scalar = "y" in variant[2:]
    piece_emit = "t" in variant[2:]  # piecewise copy+y after slice-wide stop
    fine_tail = "f" in variant[2:]  # chunks 24-29 as single-chunk DMAs
    act_copy = "a" in variant[2:]  # middle slice psum->sbuf copy on ACT

    nc = bacc.Bacc(
        "TRN2",
        target_bir_lowering=False,
        debug=False,
        enable_asserts=False,
        num_devices=N_CORES,
    )

    GS = 4
    WG = GS * NPC  # 5504
    GW = 512 + WG  # 6016: 4 xT chunks (4*128) + 4 W chunks
    HD = 768  # head: 256 cols of BB-as-f16-bytes + 512 cols xT chunks 0-3
    wx = nc.dram_tensor(
        "wx", [128, HD + K - 512 + KC * NPC], f16, kind="ExternalInput"
    ).ap()
    b = nc.dram_tensor("b", [1, NPC], f16, kind="ExternalInput").ap()
    y = nc.dram_tensor("y", [M, NPC], f16, kind="ExternalOutput").ap()

    n_slices = [(0, 512), (512, 1024), (1024, NPC)]
    TAIL0 = HD + 7 * GW  # col offset of W chunk 28

    def tailcol(kk, c0=0):
        return TAIL0 + (kk - 28) * NPC + c0

    with tile.TileContext(nc) as tc, ExitStack() as ctx:
        if loop_reps > 1:
            ctx.enter_context(tc.For_i(0, loop_reps, 1))
        singles = ctx.enter_context(tc.tile_pool(name="singles", bufs=1))
        x0pool = ctx.enter_context(tc.tile_pool(name="x0pool", bufs=2))
        wpool = ctx.enter_context(tc.tile_pool(name="wpool", bufs=8))
        tlpool = ctx.enter_context(tc.tile_pool(name="tlpool", bufs=2))
        xmtpool = ctx.enter_context(tc.tile_pool(name="xmtpool", bufs=KC))
        mrpool = ctx.enter_context(tc.tile_pool(name="mrpool", bufs=4))
        outpool = ctx.enter_context(tc.tile_pool(name="outpool", bufs=2))
        ps_y = ctx.enter_context(tc.tile_pool(name="ps_y", bufs=1, space="PSUM"))
        ps_m = ctx.enter_context(tc.tile_pool(name="ps_m", bufs=2, space="PSUM"))

        # head on sync so nothing loses DMA arbitration to the W groups;
        # BB rides as raw bytes in the f16 tile, bitcast back to f32
        head = x0pool.tile([128, HD], f16, tag="head")
        nc.sync.dma_start(out=head[:], in_=wx[:, :HD])
        bbs = head[:, :256].bitcast(f32)
        xt0 = head[:, 256:HD]
        if act_copy:
            # warmup so any ACT table load lands at the head, not the tail
            warm = singles.tile([128, 1], f16)
            nc.scalar.activation(
                out=warm[:], in_=head[:, :1],
                func=mybir.ActivationFunctionType.Copy,
            )

        if bias_nonzero:
            bias_b = singles.tile([M, NPC], f16)
            bcast = bass.AP(tensor=b.tensor, offset=b.offset, ap=[[0, M], b.ap[1]])
            nc.scalar.dma_start(out=bias_b[:], in_=bcast)

        ngroups = 6 if fine_tail else 7
        wgroups = {}
        for g in range(ngroups):
            wg_t = wpool.tile([128, GW], f16, tag="wg")
            nc.sync.dma_start(
                out=wg_t[:], in_=wx[:, HD + g * GW : HD + (g + 1) * GW]
            )
            wgroups[g] = wg_t
        wtail = {}
        if fine_tail:
            # group 6's region re-sliced: [xT(28-31) | W24] one DMA, then
            # W25..29 as singles (the host layout is unchanged)
            GRP6 = HD + 6 * GW
            t24 = tlpool.tile([128, 512 + NPC], f16, tag="t24")
            nc.sync.dma_start(out=t24[:], in_=wx[:, GRP6 : GRP6 + 512 + NPC])
            wtail[24] = t24[:, 512:]
            for kk in range(25, 30):
                wt = tlpool.tile([128, NPC], f16, tag=f"wt{kk}")
                nc.sync.dma_start(
                    out=wt[:],
                    in_=wx[
                        :, GRP6 + 512 + (kk - 24) * NPC : GRP6
                        + 512
                        + (kk - 23) * NPC
                    ],
                )
                wtail[kk] = wt[:]
        else:
            for kk in (28, 29):
                wt = tlpool.tile([128, NPC], f16, tag=f"wt{kk}")
                nc.sync.dma_start(
                    out=wt[:], in_=wx[:, tailcol(kk) : tailcol(kk + 1)]
                )
                wtail[kk] = wt[:]
        wl = {}
        for lo, hi in n_slices:
            for kk in (30, 31):
                wl_t = tlpool.tile([128, hi - lo], f16, tag=f"wl{lo}_{kk}")
                nc.sync.dma_start(
                    out=wl_t[:], in_=wx[:, tailcol(kk, lo) : tailcol(kk, hi)]
                )
                wl[(kk, lo)] = wl_t

        # xT source view for chunk group c (chunks 4c..4c+3)
        xtsrc = [xt0] + [wgroups[g][:, :512] for g in range(ngroups)]
        if fine_tail:
            xtsrc.append(t24[:, :512])

        ypsums = {}
        for i, (lo, hi) in enumerate(n_slices):
            yps_tile = ps_y.tile([M, hi - lo], f32, tag=f"ypsum{i}")
            ypsums[lo] = yps_tile
        ysb = outpool.tile([M, NPC], f16)

        def emit_out_range(pk, a, bnd):
            if bias_nonzero:
                nc.vector.tensor_tensor(
                    out=ysb[:, a:bnd],
                    in0=ypsums[pk][:, a - pk : bnd - pk],
                    in1=bias_b[:, a:bnd],
                    op=mybir.AluOpType.add,
                )
            elif act_copy and pk == 512:
                nc.scalar.activation(
                    out=ysb[:, a:bnd],
                    in_=ypsums[pk][:, a - pk : bnd - pk],
                    func=mybir.ActivationFunctionType.Copy,
                )
            else:
                nc.vector.tensor_copy(
                    out=ysb[:, a:bnd], in_=ypsums[pk][:, a - pk : bnd - pk]
                )
            eng = nc.scalar if (y_scalar or a == 512) else nc.sync
            eng.dma_start(out=y[:, a:bnd], in_=ysb[:, a:bnd])

        xmt_all = {}

        def masks_for_group(c):
            # chunks 4c..4c+3; xT in xtsrc[c]: [128, 4*128]
            xv = xtsrc[c]
            pbs = mrpool.tile([128, 32], f32, tag="pbs")
            nc.vector.tensor_reduce(
                out=pbs[:],
                in_=xv.rearrange("p (cg q) -> p cg q", q=16),
                axis=mybir.AxisListType.X,
                op=mybir.AluOpType.add,
                apply_absolute_value=True,
            )
            mask_pre = ps_m.tile([128, 32], f32)
            nc.tensor.matmul(
                mask_pre[:], lhsT=bbs, rhs=pbs[:], start=True, stop=True
            )
            maskt = mrpool.tile([128, 32], f16, tag="maskt")
            nc.vector.tensor_scalar(
                out=maskt[:],
                in0=mask_pre[:],
                scalar1=float(THRES_SUM),
                scalar2=None,
                op0=mybir.AluOpType.is_gt,
            )
            for j in range(4):
                kc = 4 * c + j
                xmt = xmtpool.tile([128, 128], f16)
                mview = maskt[:, 8 * j : 8 * j + 8].unsqueeze(2).broadcast_to(
                    [128, 8, 16]
                )
                nc.vector.tensor_tensor(
                    out=xmt[:].rearrange("p (g q) -> p g q", q=16),
                    in0=xv[:, j * 128 : (j + 1) * 128].rearrange(
                        "p (g q) -> p g q", q=16
                    ),
                    in1=mview,
                    op=mybir.AluOpType.mult,
                )
                xmt_all[kc] = xmt

        def matmuls_for(kc):
            first_single = 24 if fine_tail else 28
            if kc < first_single:
                wv = wgroups[kc // GS]
                for lo, hi in n_slices:
                    nc.tensor.matmul(
                        ypsums[lo][:],
                        lhsT=xmt_all[kc][:],
                        rhs=wv[
                            :,
                            512 + (kc % GS) * NPC + lo : 512
                            + (kc % GS) * NPC
                            + hi,
                        ],
                        start=(kc == 0),
                        stop=False,
                    )
            elif kc < 30:
                for lo, hi in n_slices:
                    nc.tensor.matmul(
                        ypsums[lo][:],
                        lhsT=xmt_all[kc][:],
                        rhs=wtail[kc][:, lo:hi],
                        start=False,
                        stop=False,
                    )
            elif kc == 31:
                for lo, hi in n_slices:
                    for kk in (30, 31):
                        nc.tensor.matmul(
                            ypsums[lo][:],
                            lhsT=xmt_all[kk][:],
                            rhs=wl[(kk, lo)][:],
                            start=False,
                            stop=(kk == 31),
                        )
                    if piece_emit and hi - lo > 256:
                        mid = lo + (hi - lo) // 2
                        emit_out_range(lo, lo, mid)
                        emit_out_range(lo, mid, hi)
                    else:
                        emit_out_range(lo, lo, hi)

        masks_for_group(0)
        for g in range(ngroups):
            masks_for_group(g + 1)
            for kc in range(4 * g, 4 * g + 4):
                matmuls_for(kc)
        if fine_tail:
            masks_for_group(7)
            for kc in range(24, 30):
                matmuls_for(kc)
        else:
            for kc in (28, 29):
                matmuls_for(kc)
        matmuls_for(31)

    nc.compile()
    return nc


def _make_in_maps_v6(x, weight, bias):
    x2d = np.asarray(x, dtype=np.float16).reshape(M, K)
    xt = np.ascontiguousarray(x2d.T)  # [K, M]; chunk kc = rows kc*128..
    wf = np.asarray(weight, dtype=np.float16)
    bf = np.asarray(bias, dtype=np.float16)
    bb = np.kron(np.eye(2, dtype=np.float32), np.ones((64, 64), np.float32))
    bb16 = np.ascontiguousarray(bb).view(np.float16)  # [128, 256] raw bytes
    xtc = xt.reshape(KC, 128, 128)  # [kc, k-part, m]
    in_maps = []
    for c in range(N_CORES):
        ws = wf[:, c * NPC : (c + 1) * NPC]
        wtiled = ws.reshape(KC, 128, NPC).transpose(1, 0, 2)  # [128, KC, NPC]
        parts = [bb16, xtc[0:4].transpose(1, 0, 2).reshape(128, 512)]
        for g in range(7):
            parts.append(
                xtc[4 * g + 4 : 4 * g + 8].transpose(1, 0, 2).reshape(128, 512)
            )
            parts.append(wtiled[:, 4 * g : 4 * g + 4, :].reshape(128, 4 * NPC))
        parts.append(wtiled[:, 28:32, :].reshape(128, 4 * NPC))
        wxm = np.ascontiguousarray(np.concatenate(parts, axis=1))
        in_maps.append(
            {
                "wx": wxm,
                "b": np.ascontiguousarray(bf[c * NPC : (c + 1) * NPC]).reshape(
                    1, NPC
                ),
            }
        )
    return in_maps


def _make_in_maps_v4(x, weight, bias):
    x2 = np.asarray(x, dtype=np.float16).reshape(M, K // 512, 512)
    wf = np.asarray(weight, dtype=np.float16)
    bf = np.asarray(bias, dtype=np.float16)
    gg = np.kron(np.eye(8, dtype=np.float32), np.ones((16, 16), np.float32))
    ident = np.eye(128, dtype=np.float16)
    in_maps = []
    for c in range(N_CORES):
        ws = wf[:, c * NPC : (c + 1) * NPC]
        wtiled = ws.reshape(KC, 128, NPC).transpose(1, 0, 2)  # [128, KC, NPC]
        parts = [x2[:, 0, :]]
        for g in range(7):
            parts.append(x2[:, g + 1, :])
            parts.append(wtiled[:, 4 * g : 4 * g + 4, :].reshape(128, 4 * NPC))
        parts.append(wtiled[:, 28:32, :].reshape(128, 4 * NPC))
        wxm = np.ascontiguousarray(np.concatenate(parts, axis=1))
        in_maps.append(
            {
                "wx": wxm,
                "b": np.ascontiguousarray(bf[c * NPC : (c + 1) * NPC]).reshape(
                    1, NPC
                ),
                "gg": gg,
                "idin": ident,
            }
        )
    return in_maps


def _make_in_maps_v3(x, weight, bias):
    x2 = np.ascontiguousarray(
        np.asarray(x, dtype=np.float16)
        .reshape(M, K // 512, 512)
        .transpose(1, 0, 2)
    )
    wf = np.asarray(weight, dtype=np.float16)
    bf = np.asarray(bias, dtype=np.float16)
    gg = np.kron(np.eye(8, dtype=np.float32), np.ones((16, 16), np.float32))
    ident = np.eye(128, dtype=np.float16)
    in_maps = []
    for c in range(N_CORES):
        ws = wf[:, c * NPC : (c + 1) * NPC]
        wtiled = np.ascontiguousarray(
            ws.reshape(KC, 128, NPC).transpose(1, 0, 2).reshape(128, KC * NPC)
        )
        in_maps.append(
            {
                "x": x2,
                "wp": wtiled,
                "b": np.ascontiguousarray(bf[c * NPC : (c + 1) * NPC]).reshape(
                    1, NPC
                ),
                "gg": gg,
                "idin": ident,
            }
        )
    return in_maps


def _make_in_maps_v2(x, weight, bias, variant="v2"):
    fused_x = variant.startswith("v2f")
    xflat = np.ascontiguousarray(np.asarray(x, dtype=np.float16).reshape(M, K))
    wf = np.asarray(weight, dtype=np.float16)
    bf = np.asarray(bias, dtype=np.float16)
    gg = np.kron(np.eye(8, dtype=np.float32), np.ones((16, 16), np.float32))
    ident = np.eye(128, dtype=np.float16)
    in_maps = []
    for c in range(N_CORES):
        ws = wf[:, c * NPC : (c + 1) * NPC]
        # partition p holds row kc*128+p of every K-chunk, chunk-major
        wtiled = np.ascontiguousarray(
            ws.reshape(KC, 128, NPC).transpose(1, 0, 2).reshape(128, KC * NPC)
        )
        if fused_x:
            wtiled = np.ascontiguousarray(
                np.concatenate([xflat, wtiled], axis=1)
            )
        m = {
            "wp": wtiled,
            "b": np.ascontiguousarray(bf[c * NPC : (c + 1) * NPC]).reshape(1, NPC),
            "gg": gg,
            "idin": ident,
        }
        if not fused_x:
            m["xf"] = xflat
        in_maps.append(m)
    return in_maps


def _get_nc(bias_nonzero: bool, loop_reps: int = 1, variant: str = ""):
    key = ("nc", bias_nonzero, loop_reps, variant)
    if key not in _STATE:
        if variant.startswith("v6"):
            _STATE[key] = _build_v6(bias_nonzero, loop_reps, variant)
        elif variant.startswith("v4") or variant.startswith("v5"):
            _STATE[key] = _build_v4(bias_nonzero, loop_reps, variant)
        elif variant.startswith("v3"):
            _STATE[key] = _build_v3(bias_nonzero, loop_reps, variant)
        elif variant.startswith("v2"):
            _STATE[key] = _build_v2(bias_nonzero, loop_reps, variant)
        else:
            _STATE[key] = _build(bias_nonzero, loop_reps, variant)
    return _STATE[key]


def _make_in_maps(x, weight, bias):
    x2 = np.ascontiguousarray(
        np.asarray(x, dtype=np.float16)
        .reshape(M, K // 512, 512)
        .transpose(1, 0, 2)
    )
    wf = np.asarray(weight, dtype=np.float16)
    bf = np.asarray(bias, dtype=np.float16)
    gg = np.kron(np.eye(8, dtype=np.float32), np.ones((16, 16), np.float32))
    ident = np.eye(128, dtype=np.float16)
    in_maps = []
    for c in range(N_CORES):
        in_maps.append(
            {
                "x": x2,
                "w": np.ascontiguousarray(wf[:, c * NPC : (c + 1) * NPC]),
                "b": np.ascontiguousarray(bf[c * NPC : (c + 1) * NPC]).reshape(
                    1, NPC
                ),
                "gg": gg,
                "idin": ident,
            }
        )
    return in_maps


DEFAULT_VARIANT = "v6f"


def kernel(x, weight, bias, _trace=False):
    from concourse.bass_utils import run_bass_kernel_spmd

    bias_nonzero = bool(np.any(np.asarray(bias)))
    nc = _get_nc(bias_nonzero, variant=DEFAULT_VARIANT)
    in_maps = _make_in_maps_v6(x, weight, bias)
    res = run_bass_kernel_spmd(
        nc, in_maps, core_ids=list(range(N_CORES)), trace=_trace
    )
    _STATE["last_results"] = res
    y = np.concatenate([res.results[c]["y"] for c in range(N_CORES)], axis=1)
    return y.reshape(M, 1, N_FULL).astype(np.float16)

